# revision 1
# baseline (speedup 1.0000x reference)
"""Trainium2 Bass kernel for 2-layer GAT (nn_GAT_84146999263862).

Strategy (8 NeuronCores, SPMD):
  - Nodes padded to NP=50176 = 8*49*128; core c owns node slice [c*6272,(c+1)*6272).
  - Edges (plus self-loops) are assigned to cores by dst slice, grouped per
    128-node dst block, packed into 128-edge chunks (dummy pad edges get
    dst_local=255 so their one-hot column is all-zero).
  - Per-edge rows are fetched with gpsimd.dma_gather (int16 indices => tables
    split lo/hi at row 32768; chunks grouped by src half; per-core counts are
    padded to a shared static graph).
  - Per chunk: is_equal one-hot matrices map edges<->dst slots; PE matmuls do
    adst expansion and segment reduction (messages + softmax denominators
    accumulate in PSUM per 128-node block). Softmax skips max-subtraction
    (logits are O(1); mathematically identical).
  - Each core builds its z1 table locally in ROTATED node order (own nodes at
    rows 0..SLICE) so adst/h1 addresses are static; gather indices compensate.
  - One small AllGather shares z2_ext between layers.

Host does integer-only preprocessing (sorting/packing/index maps and pure data
movement like transposes); all floating-point math runs on device.
"""
import sys
import numpy as np

sys.path.insert(0, '/opt/trn_rl_repo')

from contextlib import ExitStack
from concourse import bass, bacc, mybir, tile, library_config
from concourse.bass_utils import run_bass_kernel_spmd
from concourse.masks import make_identity
from concourse.tile import ScopedClock

# This walrus build rejects multi-wait TPB_CTRL instructions; split the Tile
# tail-drain's semaphore waits across single-wait drains.
_MAXW = 1


def _patched_drain_and_barrier(self, tick_clock, wait_clock):
    drain = self.nc.sync.drain()
    wait_clock.add_sem_waits(drain.ins,
                             ScopedClock({None: tick_clock.global_clock}))
    si = drain.ins.sync_info
    waits = list(si.on_wait)
    if len(waits) > _MAXW:
        si.on_wait = waits[:_MAXW]
        for k in range(_MAXW, len(waits), _MAXW):
            extra = self.nc.sync.drain()
            extra.ins.sync_info = mybir.SyncInfo(on_wait=waits[k:k+_MAXW],
                                                 on_update=[])
    self.nc.all_engine_barrier()
    popped = self.nc._tile_sem_poison_stack.pop()
    assert popped is self._sem_poison
    self.nc.clear_and_free_semaphores(list(self.sems.allocated().values()))
    self.nc.all_engine_barrier()


tile.TileContext._drain_and_barrier = _patched_drain_and_barrier

F32 = mybir.dt.float32
I16 = mybir.dt.int16
NEG = 0.2
BLK = 128
GG = 8                     # chunks per dma_gather instruction (1024 indices)
MG = 4                     # chunks per metarow-broadcast matmul (512 cols)


class Cfg:
    def __init__(self, N=50000, cores=8, bpc=49, half=32768,
                 f_in=256, heads=8, ch=32, cls_=32):
        self.N = N
        self.CORES = cores
        self.BPC = bpc
        self.SLICE = bpc * BLK
        self.NP = cores * self.SLICE
        self.HALF = half
        self.F_IN = f_in
        self.HEADS = heads
        self.CH = ch
        self.D1 = heads * ch
        self.CLS = cls_
        self.TAB1_W = 320 if self.D1 == 256 else self.D1 + 64   # row: z|asrc|adst|pad
        self.TAB2_W = 64                                        # z2|asrc2|adst2|pad
        assert self.TAB1_W * 4 % 256 == 0 and self.TAB2_W * 4 % 256 == 0
        assert self.HALF % BLK == 0 and self.HALF < 32768 + 1
        assert self.NP - self.HALF <= 32767


FULL = Cfg()


# ---------------------------------------------------------------- host side

def _wrap16(vals):
    """[1024] ints -> [128, 64] int16 (wrapped 16 partitions, replicated x8)."""
    v = np.asarray(vals, np.int64).reshape(64, 16)
    arr = np.zeros((128, 64), np.int16)
    arr[:16, :] = v.T
    for r in range(1, 8):
        arr[r*16:(r+1)*16] = arr[:16]
    return arr


def host_prep(edge_index, cfg):
    """Integer-only preprocessing. Returns (counts, per_core_arrays)."""
    src = np.asarray(edge_index[0], np.int64)
    dst = np.asarray(edge_index[1], np.int64)
    loops = np.arange(cfg.N, dtype=np.int64)
    src = np.concatenate([src, loops])
    dst = np.concatenate([dst, loops])

    core = dst // cfg.SLICE
    blk_in_core = (dst % cfg.SLICE) // BLK
    dst_local = dst % BLK

    def chunkify(iv, dl):
        out = []
        for i in range(0, len(iv), BLK):
            a, b = iv[i:i+BLK], dl[i:i+BLK]
            pad = BLK - len(a)
            if pad:
                a = np.concatenate([a, np.zeros(pad, np.int64)])
                b = np.concatenate([b, np.full(pad, 255, np.int64)])
            out.append((a, b))
        return out

    ch = {1: {}, 2: {}}
    for c in range(cfg.CORES):
        m_c = core == c
        s_c, dl_c, bi_c = src[m_c], dst_local[m_c], blk_in_core[m_c]
        rot = (s_c - cfg.SLICE * c) % cfg.NP
        for layer, ids in ((1, rot), (2, s_c)):
            lo = ids < cfg.HALF
            for i in range(cfg.BPC):
                m_b = bi_c == i
                for grp in range(2):
                    m = m_b & (lo if grp == 0 else ~lo)
                    iv = ids[m] - (0 if grp == 0 else cfg.HALF)
                    ch[layer][(c, i, grp)] = chunkify(iv, dl_c[m])

    counts = {}
    for layer in (1, 2):
        nmax = np.zeros((cfg.BPC, 2), np.int64)
        for (c, i, g), lst in ch[layer].items():
            nmax[i, g] = max(nmax[i, g], len(lst))
        counts[layer] = nmax

    per_core = []
    for c in range(cfg.CORES):
        data = {}
        for layer in (1, 2):
            nmax = counts[layer]
            all_chunks = []
            for i in range(cfg.BPC):
                for g in range(2):
                    lst = ch[layer][(c, i, g)]
                    for k in range(int(nmax[i, g])):
                        if k < len(lst):
                            iv, dl = lst[k]
                        else:
                            iv = np.zeros(BLK, np.int64)
                            dl = np.full(BLK, 255, np.int64)
                        all_chunks.append((g, iv, dl))
            totch = len(all_chunks)
            stream = [[], []]
            for g, iv, dl in all_chunks:
                stream[g].append(iv)
            for g in range(2):
                s = stream[g]
                while len(s) % GG:
                    s.append(np.zeros(BLK, np.int64))
                ng = max(1, len(s) // GG)
                arr = np.zeros((128, ng * 64), np.int16)
                for gi in range(len(s) // GG):
                    arr[:, gi*64:(gi+1)*64] = _wrap16(
                        np.concatenate(s[gi*GG:(gi+1)*GG]))
                data[('idxlo' if g == 0 else 'idxhi') + str(layer)] = arr
            metacol = np.zeros((128, totch), np.float32)
            metarow = np.zeros((1, totch * BLK), np.float32)
            for j, (g, iv, dl) in enumerate(all_chunks):
                metacol[:, j] = dl
                metarow[0, j*BLK:(j+1)*BLK] = dl
            data[f'metacol{layer}'] = metacol
            data[f'metarow{layer}'] = metarow
        per_core.append(data)
    return counts, per_core


def host_weights(inputs, cfg):
    """Weight/constant staging (reordering + transposes only, no math)."""
    W1 = np.asarray(inputs['W1'], np.float32)
    a_src1 = np.asarray(inputs['a_src1'], np.float32)
    a_dst1 = np.asarray(inputs['a_dst1'], np.float32)
    b1 = np.asarray(inputs['b1'], np.float32)
    W2 = np.asarray(inputs['W2'], np.float32)
    a_src2 = np.asarray(inputs['a_src2'], np.float32)
    a_dst2 = np.asarray(inputs['a_dst2'], np.float32)
    b2 = np.asarray(inputs['b2'], np.float32)

    H, C, D1 = cfg.HEADS, cfg.CH, cfg.D1
    perm = np.empty(D1, np.int64)
    for h in range(H):
        for c_ in range(C):
            perm[c_*H + h] = h*C + c_
    consts = {
        'W1cm': W1[:, perm].copy(), 'W1T': W1.T.copy(),
        'a_src1': a_src1, 'a_dst1': a_dst1, 'b1cm': b1[perm][None, :].copy(),
        'W2p': W2[perm, :].copy(), 'W2pT': W2[perm, :].T.copy(),
        'a_src2': a_src2, 'a_dst2': a_dst2, 'b2': b2[None, :].copy(),
        'iota_row': np.broadcast_to(np.arange(128, dtype=np.float32),
                                    (128, 128)).copy(),
        'iota_col': np.arange(128, dtype=np.float32)[:, None].copy(),
        'ones_row': np.ones((1, 128), np.float32),
    }
    x = np.asarray(inputs['x'], np.float32)
    xpad = np.zeros((cfg.NP, cfg.F_IN), np.float32)
    xpad[:cfg.N] = x
    xT_rots = [np.roll(xpad, -cfg.SLICE * c, axis=0).T.copy()
               for c in range(cfg.CORES)]
    return consts, xT_rots


# ---------------------------------------------------------------- device side

def build_gat(counts, cfg):
    nc = bacc.Bacc()
    H, C, D1, CLS, F_IN = cfg.HEADS, cfg.CH, cfg.D1, cfg.CLS, cfg.F_IN
    T1, T2 = cfg.TAB1_W, cfg.TAB2_W
    E1 = D1 + 2 * H          # written z1 table cols (z | asrc | adst)
    FH = F_IN // 128

    def n_stream(layer, g):
        return max(1, -(-int(counts[layer][:, g].sum()) // GG))

    GLO1, GHI1 = n_stream(1, 0), n_stream(1, 1)
    GLO2, GHI2 = n_stream(2, 0), n_stream(2, 1)
    TOT1, TOT2 = int(counts[1].sum()), int(counts[2].sum())

    inp = {}
    for name, shape, dt in [
        ('xT_rot', [F_IN, cfg.NP], F32),
        ('W1cm', [F_IN, D1], F32), ('W1T', [D1, F_IN], F32),
        ('a_src1', [H, C], F32), ('a_dst1', [H, C], F32),
        ('b1cm', [1, D1], F32),
        ('W2p', [D1, CLS], F32), ('W2pT', [CLS, D1], F32),
        ('a_src2', [1, CLS], F32), ('a_dst2', [1, CLS], F32),
        ('b2', [1, CLS], F32),
        ('iota_row', [128, 128], F32), ('iota_col', [128, 1], F32),
        ('ones_row', [1, 128], F32),
        ('idxlo1', [128, GLO1 * 64], I16), ('idxhi1', [128, GHI1 * 64], I16),
        ('idxlo2', [128, GLO2 * 64], I16), ('idxhi2', [128, GHI2 * 64], I16),
        ('metacol1', [128, TOT1], F32), ('metarow1', [1, TOT1 * BLK], F32),
        ('metacol2', [128, TOT2], F32), ('metarow2', [1, TOT2 * BLK], F32),
    ]:
        inp[name] = nc.declare_dram_parameter(name, shape, dt, isOutput=False)

    out_d = nc.declare_dram_parameter('out', [cfg.SLICE, CLS], F32, isOutput=True)

    z1tab = nc.dram_tensor('z1tab', [cfg.NP, T1], F32)
    h1loc = nc.dram_tensor('h1loc', [cfg.SLICE, D1], F32)
    z2slice = nc.dram_tensor('z2slice', [cfg.SLICE, T2], F32)
    z2cat = nc.dram_tensor('z2cat', [cfg.NP, T2], F32)

    with tile.TileContext(nc) as tc, ExitStack() as ctx:
        sb = ctx.enter_context(tc.tile_pool(name='sb', bufs=1))
        sbw = ctx.enter_context(tc.tile_pool(name='sbw', bufs=2))

        nc.gpsimd.load_library(library_config.mlp)

        ident = sb.tile([128, 128], F32)
        make_identity(nc, ident[:])
        iota_r = sb.tile([128, 128], F32)
        nc.sync.dma_start(out=iota_r[:], in_=inp['iota_row'][:, :])
        iota_c = sb.tile([128, 1], F32)
        nc.sync.dma_start(out=iota_c[:], in_=inp['iota_col'][:, :])
        ones_r = sb.tile([1, 128], F32)
        nc.sync.dma_start(out=ones_r[:], in_=inp['ones_row'][:, :])

        W1e = [sb.tile([128, T1], F32, tag=f'w1e{_i}', name=f'W1e{_i}') for _i in range(FH)]
        W2e = [sb.tile([128, T2], F32, tag=f'w2e{_i}', name=f'W2e{_i}') for _i in range(FH)]
        b1b = sb.tile([128, D1], F32)
        b2b = sb.tile([128, CLS], F32)

        with tc.tile_pool(name='p0sb', bufs=1) as p0sb, \
             tc.tile_pool(name='p0ps', bufs=1, space='PSUM') as p0ps:
            for fh in range(FH):
                nc.vector.memset(W1e[fh][:], 0.0)
                nc.vector.memset(W2e[fh][:], 0.0)
            # ---- W1_ext = [W1cm | W1@A_src | W1@A_dst]
            a1 = p0sb.tile([H, 2 * C], F32)
            nc.sync.dma_start(out=a1[:, 0:C], in_=inp['a_src1'][:, :])
            nc.sync.dma_start(out=a1[:, C:2*C], in_=inp['a_dst1'][:, :])
            a1T_ps = p0ps.tile([128, 128], F32, space='PSUM', tag='t')
            nc.tensor.transpose(out=a1T_ps[0:2*C, 0:H], in_=a1[:], identity=ident[0:H, 0:H])
            a1T = p0sb.tile([2 * C, H], F32)
            nc.vector.tensor_copy(out=a1T[:], in_=a1T_ps[0:2*C, 0:H])
            A_bd = p0sb.tile([128, FH, 2 * H], F32)
            nc.vector.memset(A_bd[:], 0.0)
            for h in range(H):
                half, off = divmod(h * C, 128)
                nc.vector.tensor_copy(out=A_bd[off:off+C, half, h:h+1],
                                      in_=a1T[0:C, h:h+1])
                nc.vector.tensor_copy(out=A_bd[off:off+C, half, H+h:H+h+1],
                                      in_=a1T[C:2*C, h:h+1])
            w1t_sb = [p0sb.tile([128, F_IN], F32, tag=f'w1t{_i}', name=f'w1t{_i}') for _i in range(FH)]
            for cc in range(FH):
                nc.sync.dma_start(out=w1t_sb[cc][:],
                                  in_=inp['W1T'][cc*128:(cc+1)*128, :])
            w1a_ps = p0ps.tile([128, FH, 2 * H], F32, space='PSUM', tag='a')
            for fh in range(FH):
                for cc in range(FH):
                    nc.tensor.matmul(out=w1a_ps[:, fh, :],
                                     lhsT=w1t_sb[cc][:, fh*128:(fh+1)*128],
                                     rhs=A_bd[:, cc, :],
                                     start=(cc == 0), stop=(cc == FH - 1))
            for fh in range(FH):
                nc.sync.dma_start(out=W1e[fh][:, 0:D1],
                                  in_=inp['W1cm'][fh*128:(fh+1)*128, :])
                nc.vector.tensor_copy(out=W1e[fh][:, D1:D1+2*H],
                                      in_=w1a_ps[:, fh, :])

            # ---- W2_ext = [W2p | W2p@a_src2^T | W2p@a_dst2^T]
            a2 = p0sb.tile([2, CLS], F32)
            nc.sync.dma_start(out=a2[0:1, :], in_=inp['a_src2'][:, :])
            nc.sync.dma_start(out=a2[1:2, :], in_=inp['a_dst2'][:, :])
            a2T_ps = p0ps.tile([128, 128], F32, space='PSUM', tag='t')
            nc.tensor.transpose(out=a2T_ps[0:CLS, 0:2], in_=a2[:], identity=ident[0:2, 0:2])
            a2T = p0sb.tile([CLS, 2], F32)
            nc.vector.tensor_copy(out=a2T[:], in_=a2T_ps[0:CLS, 0:2])
            w2t_sb = p0sb.tile([CLS, D1], F32)
            nc.sync.dma_start(out=w2t_sb[:], in_=inp['W2pT'][:, :])
            w2a_ps = p0ps.tile([128, FH, 2], F32, space='PSUM', tag='a')
            for fh in range(FH):
                nc.tensor.matmul(out=w2a_ps[:, fh, :],
                                 lhsT=w2t_sb[:, fh*128:(fh+1)*128],
                                 rhs=a2T[:], start=True, stop=True)
            for fh in range(FH):
                nc.sync.dma_start(out=W2e[fh][:, 0:CLS],
                                  in_=inp['W2p'][fh*128:(fh+1)*128, :])
                nc.vector.tensor_copy(out=W2e[fh][:, CLS:CLS+2],
                                      in_=w2a_ps[:, fh, :])

            # ---- bias broadcast tiles
            b1_sb = p0sb.tile([1, D1], F32)
            nc.sync.dma_start(out=b1_sb[:], in_=inp['b1cm'][:, :])
            b1b_ps = p0ps.tile([128, D1], F32, space='PSUM', tag='b')
            nc.tensor.matmul(out=b1b_ps[:], lhsT=ones_r[:], rhs=b1_sb[:],
                             start=True, stop=True)
            nc.vector.tensor_copy(out=b1b[:], in_=b1b_ps[:])
            b2_sb = p0sb.tile([1, CLS], F32)
            nc.sync.dma_start(out=b2_sb[:], in_=inp['b2'][:, :])
            b2b_ps = p0ps.tile([128, CLS], F32, space='PSUM', tag='b')
            nc.tensor.matmul(out=b2b_ps[:], lhsT=ones_r[:], rhs=b2_sb[:],
                             start=True, stop=True)
            nc.vector.tensor_copy(out=b2b[:], in_=b2b_ps[:])

        # ---- P1: z1 table build (rotated order)
        with tc.tile_pool(name='p1sb', bufs=3) as p1sb, \
             tc.tile_pool(name='p1ps', bufs=2, space='PSUM') as p1ps:
            for t in range(cfg.NP // 128):
                zps = p1ps.tile([128, T1], F32, space='PSUM')
                for fh in range(FH):
                    xt = p1sb.tile([128, 128], F32, tag='xt')
                    nc.sync.dma_start(
                        out=xt[:],
                        in_=inp['xT_rot'][fh*128:(fh+1)*128, t*128:(t+1)*128])
                    nc.tensor.matmul(out=zps[:], lhsT=xt[:], rhs=W1e[fh][:],
                                     start=(fh == 0), stop=(fh == FH - 1))
                zsb = p1sb.tile([128, T1], F32, tag='zsb')
                if t % 2 == 0:
                    nc.vector.tensor_copy(out=zsb[:], in_=zps[:])
                else:
                    nc.scalar.activation(zsb[:], zps[:],
                                         mybir.ActivationFunctionType.Copy)
                nc.sync.dma_start(out=z1tab[t*128:(t+1)*128, :], in_=zsb[:])

        # ---- edge phase (shared between layers)
        def edge_phase(layer, tab_lo, tab_hi, tabw, zcols, heads,
                       adst_ap, finalize):
            nmax = counts[layer]
            metarow_d = inp[f'metarow{layer}']
            tot = int(nmax.sum())

            with tc.tile_pool(name=f'ep{layer}', bufs=1) as ep, \
                 tc.tile_pool(name=f'em{layer}', bufs=3) as em, \
                 tc.tile_pool(name=f'eg{layer}', bufs=3) as eg, \
                 tc.tile_pool(name=f'ew{layer}', bufs=3) as ew, \
                 tc.tile_pool(name=f'el{layer}', bufs=2, space='PSUM') as el, \
                 tc.tile_pool(name=f'ea{layer}', bufs=2, space='PSUM') as ea:

                idx_lo = ep.tile([128, inp[f'idxlo{layer}'].shape[1]], I16)
                nc.sync.dma_start(out=idx_lo[:], in_=inp[f'idxlo{layer}'][:, :])
                idx_hi = ep.tile([128, inp[f'idxhi{layer}'].shape[1]], I16)
                nc.sync.dma_start(out=idx_hi[:], in_=inp[f'idxhi{layer}'][:, :])
                metacol = ep.tile([128, tot], F32)
                nc.sync.dma_start(out=metacol[:], in_=inp[f'metacol{layer}'][:, :])

                gtiles = [{}, {}]
                stream_pos = [0, 0]
                mtiles = {}
                chunk_idx = 0
                for i in range(cfg.BPC):
                    nblk = int(nmax[i, 0] + nmax[i, 1])
                    if nblk == 0:
                        continue
                    adst_blk = ew.tile([128, heads], F32, tag='adst')
                    nc.sync.dma_start(out=adst_blk[:], in_=adst_ap(i))
                    accps = ea.tile([128, zcols + heads], F32, space='PSUM',
                                    tag='acc')
                    done = 0
                    for g in range(2):
                        for _k in range(int(nmax[i, g])):
                            pos = stream_pos[g]
                            stream_pos[g] += 1
                            gi, j = divmod(pos, GG)
                            if gi not in gtiles[g]:
                                gt = eg.tile([128, GG, tabw], F32, tag=f'g{g}')
                                nc.gpsimd.dma_gather(
                                    out_ap=gt[:],
                                    in_ap=tab_lo if g == 0 else tab_hi,
                                    idxs_ap=(idx_lo if g == 0 else idx_hi)[
                                        :, gi*64:(gi+1)*64],
                                    num_idxs=GG * BLK, num_idxs_reg=GG * BLK,
                                    elem_size=tabw)
                                gtiles[g][gi] = gt
                                for old in [k_ for k_ in gtiles[g]
                                            if k_ < gi - 2]:
                                    del gtiles[g][old]
                            gt = gtiles[g][gi]

                            mg, mj = divmod(chunk_idx, MG)
                            if mg not in mtiles:
                                lo = mg * MG * BLK
                                hi = min(tot * BLK, lo + MG * BLK)
                                mrow = em.tile([1, MG * BLK], F32, tag='mrow')
                                nc.sync.dma_start(out=mrow[0:1, 0:hi-lo],
                                                  in_=metarow_d[0:1, lo:hi])
                                mb_ps = el.tile([128, MG * BLK], F32,
                                                space='PSUM', tag='mb')
                                nc.tensor.matmul(out=mb_ps[:, 0:hi-lo],
                                                 lhsT=ones_r[:],
                                                 rhs=mrow[0:1, 0:hi-lo],
                                                 start=True, stop=True)
                                mtiles[mg] = mb_ps
                                for old in [k_ for k_ in mtiles
                                            if k_ < mg - 1]:
                                    del mtiles[old]
                            mb_ps = mtiles[mg]

                            onehot = ew.tile([128, 128], F32, tag='oh')
                            nc.vector.tensor_scalar(
                                out=onehot[:], in0=iota_r[:],
                                scalar1=metacol[:, chunk_idx:chunk_idx+1],
                                scalar2=None, op0=mybir.AluOpType.is_equal)
                            onehotT = ew.tile([128, 128], F32, tag='ohT')
                            nc.vector.tensor_scalar(
                                out=onehotT[:],
                                in0=mb_ps[:, mj*BLK:(mj+1)*BLK],
                                scalar1=iota_c[:, 0:1],
                                scalar2=None, op0=mybir.AluOpType.is_equal)

                            lg_ps = el.tile([128, heads], F32, space='PSUM',
                                            tag='lg')
                            nc.tensor.matmul(out=lg_ps[:], lhsT=onehotT[:],
                                             rhs=adst_blk[:],
                                             start=True, stop=False)
                            nc.tensor.matmul(out=lg_ps[:], lhsT=ident[:],
                                             rhs=gt[:, j, zcols:zcols+heads],
                                             start=False, stop=True)
                            lg02 = ew.tile([128, heads], F32, tag='lg02')
                            nc.vector.tensor_scalar(
                                out=lg02[:], in0=lg_ps[:], scalar1=NEG,
                                scalar2=None, op0=mybir.AluOpType.mult)
                            lrl = ew.tile([128, heads], F32, tag='lrl')
                            nc.vector.tensor_tensor(
                                out=lrl[:], in0=lg_ps[:], in1=lg02[:],
                                op=mybir.AluOpType.max)
                            w_t = ew.tile([128, heads], F32, tag='wt')
                            nc.scalar.activation(w_t[:], lrl[:],
                                                 mybir.ActivationFunctionType.Exp)

                            smsg = ew.tile([128, zcols], F32, tag='smsg')
                            if heads > 1:
                                nc.vector.tensor_tensor(
                                    out=smsg[:], in0=gt[:, j, 0:zcols],
                                    in1=w_t[:, None, :].to_broadcast(
                                        [128, zcols // heads, heads]),
                                    op=mybir.AluOpType.mult)
                            else:
                                nc.vector.tensor_scalar(
                                    out=smsg[:], in0=gt[:, j, 0:zcols],
                                    scalar1=w_t[:, 0:1], scalar2=None,
                                    op0=mybir.AluOpType.mult)

                            nc.tensor.matmul(out=accps[:, 0:zcols],
                                             lhsT=onehot[:], rhs=smsg[:],
                                             start=(done == 0), stop=False)
                            nc.tensor.matmul(out=accps[:, zcols:zcols+heads],
                                             lhsT=onehot[:], rhs=w_t[:],
                                             start=False,
                                             stop=(done == nblk - 1))
                            done += 1
                            chunk_idx += 1
                    finalize(i, accps)

        # ---- L1 finalize: normalize + bias + elu -> h1loc
        def fin1(i, accps):
            den = sbw.tile([128, H], F32, tag='den')
            nc.vector.tensor_scalar(out=den[:], in0=accps[:, D1:D1+H],
                                    scalar1=1e-30, scalar2=None,
                                    op0=mybir.AluOpType.max)
            rec = sbw.tile([128, H], F32, tag='rec')
            nc.vector.reciprocal(out=rec[:], in_=den[:])
            h1t = sbw.tile([128, D1], F32, tag='h1t')
            nc.vector.tensor_tensor(
                out=h1t[:], in0=accps[:, 0:D1],
                in1=rec[:, None, :].to_broadcast([128, D1 // H, H]),
                op=mybir.AluOpType.mult)
            h1c = sbw.tile([128, D1], F32, tag='h1c')
            nc.vector.tensor_tensor(out=h1c[:], in0=h1t[:], in1=b1b[:],
                                    op=mybir.AluOpType.add)
            # elu(x) = exp(min(x,0)) - 1 + max(x,0)
            m0 = sbw.tile([128, D1], F32, tag='m0')
            nc.vector.tensor_scalar(out=m0[:], in0=h1c[:], scalar1=0.0,
                                    scalar2=None, op0=mybir.AluOpType.min)
            ex = sbw.tile([128, D1], F32, tag='ex')
            nc.scalar.activation(ex[:], m0[:], mybir.ActivationFunctionType.Exp)
            rl = sbw.tile([128, D1], F32, tag='rl')
            nc.vector.tensor_scalar(out=rl[:], in0=h1c[:], scalar1=0.0,
                                    scalar2=None, op0=mybir.AluOpType.max)
            h1f = sbw.tile([128, D1], F32, tag='h1f')
            nc.vector.tensor_tensor(out=h1f[:], in0=ex[:], in1=rl[:],
                                    op=mybir.AluOpType.add)
            h1o = sbw.tile([128, D1], F32, tag='h1o')
            nc.vector.tensor_scalar(out=h1o[:], in0=h1f[:], scalar1=-1.0,
                                    scalar2=None, op0=mybir.AluOpType.add)
            nc.sync.dma_start(out=h1loc[i*128:(i+1)*128, :], in_=h1o[:])

        edge_phase(1, z1tab[0:cfg.HALF, :], z1tab[cfg.HALF:, :], T1, D1, H,
                   lambda i: z1tab[i*128:(i+1)*128, D1+H:D1+2*H], fin1)

        # ---- P3: z2slice = [h1 @ W2p | asrc2 | adst2]
        with tc.tile_pool(name='p3sb', bufs=3) as p3sb, \
             tc.tile_pool(name='p3ps', bufs=2, space='PSUM') as p3ps:
            for t in range(cfg.BPC):
                h1tile = p3sb.tile([128, D1], F32, tag='h1')
                nc.sync.dma_start(out=h1tile[:], in_=h1loc[t*128:(t+1)*128, :])
                zps = p3ps.tile([128, T2], F32, space='PSUM', tag='z')
                for fh in range(FH):
                    tp = p3ps.tile([128, 128], F32, space='PSUM', tag='tp')
                    nc.tensor.transpose(out=tp[:],
                                        in_=h1tile[:, fh*128:(fh+1)*128],
                                        identity=ident[:])
                    h1T = p3sb.tile([128, 128], F32, tag='h1T')
                    if fh % 2 == 0:
                        nc.vector.tensor_copy(out=h1T[:], in_=tp[:])
                    else:
                        nc.scalar.activation(h1T[:], tp[:],
                                             mybir.ActivationFunctionType.Copy)
                    nc.tensor.matmul(out=zps[:], lhsT=h1T[:], rhs=W2e[fh][:],
                                     start=(fh == 0), stop=(fh == FH - 1))
                zsb = p3sb.tile([128, T2], F32, tag='zsb')
                nc.vector.tensor_copy(out=zsb[:], in_=zps[:])
                nc.sync.dma_start(out=z2slice[t*128:(t+1)*128, :], in_=zsb[:])

        # ---- P4: AllGather z2slice -> z2cat
        nc.gpsimd.collective_compute(
            'AllGather', mybir.AluOpType.bypass,
            replica_groups=[list(range(cfg.CORES))],
            ins=[z2slice.ap().opt()],
            outs=[z2cat.ap().opt()])

        # ---- L2 finalize: normalize + bias -> out
        def fin2(i, accps):
            den = sbw.tile([128, 1], F32, tag='den2')
            nc.vector.tensor_scalar(out=den[:], in0=accps[:, CLS:CLS+1],
                                    scalar1=1e-30, scalar2=None,
                                    op0=mybir.AluOpType.max)
            rec2 = sbw.tile([128, 1], F32, tag='rec2')
            nc.vector.reciprocal(out=rec2[:], in_=den[:])
            o1 = sbw.tile([128, CLS], F32, tag='o1')
            nc.vector.tensor_scalar(out=o1[:], in0=accps[:, 0:CLS],
                                    scalar1=rec2[:, 0:1], scalar2=None,
                                    op0=mybir.AluOpType.mult)
            o2 = sbw.tile([128, CLS], F32, tag='o2')
            nc.vector.tensor_tensor(out=o2[:], in0=o1[:], in1=b2b[:],
                                    op=mybir.AluOpType.add)
            nc.sync.dma_start(out=out_d[i*128:(i+1)*128, :], in_=o2[:])

        edge_phase(2, z2cat[0:cfg.HALF, :], z2cat[cfg.HALF:, :], T2, CLS, 1,
                   lambda i: z2slice[i*128:(i+1)*128, CLS+1:CLS+2], fin2)

    return nc


# ---------------------------------------------------------------- entry point

def run(inputs, cfg, sim=False):
    counts, per_core = host_prep(inputs['edge_index'], cfg)
    consts, xT_rots = host_weights(inputs, cfg)
    nc = build_gat(counts, cfg)
    in_maps = []
    for c in range(cfg.CORES):
        m = dict(consts)
        m['xT_rot'] = xT_rots[c]
        m.update(per_core[c])
        in_maps.append(m)
    if not sim:
        nc.compile()
    if sim:
        from concourse import bass_interp
        ms = bass_interp.MultiCoreSim(nc, cfg.CORES,
                                      num_workers=min(8, cfg.CORES))
        for c in range(cfg.CORES):
            for k, v in in_maps[c].items():
                ms.cores[c].tensor(k)[:] = v
        ms.simulate()
        outs = [np.array(ms.cores[c].mem_tensor('out')).reshape(cfg.SLICE, cfg.CLS)
                for c in range(cfg.CORES)]
    else:
        res = run_bass_kernel_spmd(nc, in_maps, core_ids=list(range(cfg.CORES)))
        outs = [np.asarray(res.results[c]['out']).reshape(cfg.SLICE, cfg.CLS)
                for c in range(cfg.CORES)]
    full = np.concatenate(outs, 0)
    return full[:cfg.N].astype(np.float32)


def kernel(**inputs):
    return run(inputs, FULL, sim=False)



# revision 20
# speedup vs baseline: 1.5807x; 1.5807x over previous
"""Trainium2 Bass kernel for 2-layer GAT (nn_GAT_84146999263862).

Strategy (8 NeuronCores, SPMD):
  - Nodes padded to NP=50176 = 8*49*128; core c owns node slice [c*6272,(c+1)*6272).
  - Edges (plus self-loops) are assigned to cores by dst slice, grouped per
    128-node dst block, packed into 128-edge chunks (dummy pad edges get
    dst_local=255 so their one-hot column is all-zero).
  - Per-edge rows are fetched with gpsimd.dma_gather (int16 indices => tables
    split lo/hi at row 32768; chunks grouped by src half; per-core counts are
    padded to a shared static graph).
  - Per chunk: is_equal one-hot matrices map edges<->dst slots; PE matmuls do
    adst expansion and segment reduction (messages + softmax denominators
    accumulate in PSUM per 128-node block). Softmax skips max-subtraction
    (logits are O(1); mathematically identical).
  - Each core builds its z1 table locally in ROTATED node order (own nodes at
    rows 0..SLICE) so adst/h1 addresses are static; gather indices compensate.
  - One small AllGather shares z2_ext between layers.

Host does integer-only preprocessing (sorting/packing/index maps and pure data
movement like transposes); all floating-point math runs on device.
"""
import sys
import numpy as np

sys.path.insert(0, '/opt/trn_rl_repo')

from contextlib import ExitStack
from concourse import bass, bacc, mybir, tile, library_config
from concourse.bass_utils import run_bass_kernel_spmd
from concourse.masks import make_identity
from concourse.tile import ScopedClock

# This walrus build rejects multi-wait TPB_CTRL instructions; split the Tile
# tail-drain's semaphore waits across single-wait drains.
_MAXW = 1


def _patched_drain_and_barrier(self, tick_clock, wait_clock):
    drain = self.nc.sync.drain()
    wait_clock.add_sem_waits(drain.ins,
                             ScopedClock({None: tick_clock.global_clock}))
    si = drain.ins.sync_info
    waits = list(si.on_wait)
    if len(waits) > _MAXW:
        si.on_wait = waits[:_MAXW]
        for k in range(_MAXW, len(waits), _MAXW):
            extra = self.nc.sync.drain()
            extra.ins.sync_info = mybir.SyncInfo(on_wait=waits[k:k+_MAXW],
                                                 on_update=[])
    self.nc.all_engine_barrier()
    popped = self.nc._tile_sem_poison_stack.pop()
    assert popped is self._sem_poison
    self.nc.clear_and_free_semaphores(list(self.sems.allocated().values()))
    self.nc.all_engine_barrier()


tile.TileContext._drain_and_barrier = _patched_drain_and_barrier

F32 = mybir.dt.float32
BF16 = mybir.dt.bfloat16
I16 = mybir.dt.int16
NEG = 0.2
BLK = 128
GG = 8                     # chunks per dma_gather instruction (1024 indices)
MG = 4                     # chunks per metarow-broadcast matmul (512 cols)


class Cfg:
    def __init__(self, N=50000, cores=8, bpc=49, half=32768,
                 f_in=256, heads=8, ch=32, cls_=32):
        self.N = N
        self.CORES = cores
        self.BPC = bpc
        self.SLICE = bpc * BLK
        self.NP = cores * self.SLICE
        self.HALF = half
        self.F_IN = f_in
        self.HEADS = heads
        self.CH = ch
        self.D1 = heads * ch
        self.CLS = cls_
        # bf16 tables: row byte size must be a multiple of 256 (dma_gather)
        self.TAB1_W = 384                  # row: z(256)|asrc(8)|adst(8)|pad
        self.TAB2_W = 128                  # z2(32)|asrc2|adst2|pad
        assert self.TAB1_W * 2 % 256 == 0 and self.TAB2_W * 2 % 256 == 0
        assert self.HALF % BLK == 0 and self.HALF < 32768 + 1
        assert self.NP - self.HALF <= 32767


FULL = Cfg()


# ---------------------------------------------------------------- host side

def _wrap16(vals):
    """[1024] ints -> [128, 64] int16 (wrapped 16 partitions, replicated x8)."""
    v = np.asarray(vals, np.int64).reshape(64, 16)
    arr = np.zeros((128, 64), np.int16)
    arr[:16, :] = v.T
    for r in range(1, 8):
        arr[r*16:(r+1)*16] = arr[:16]
    return arr


def host_prep(edge_index, cfg):
    """Integer-only preprocessing. Returns (counts, per_core_arrays)."""
    src = np.asarray(edge_index[0], np.int64)
    dst = np.asarray(edge_index[1], np.int64)
    loops = np.arange(cfg.N, dtype=np.int64)
    src = np.concatenate([src, loops])
    dst = np.concatenate([dst, loops])

    core = dst // cfg.SLICE
    blk_in_core = (dst % cfg.SLICE) // BLK
    dst_local = dst % BLK

    def chunkify(iv, dl):
        out = []
        for i in range(0, len(iv), BLK):
            a, b = iv[i:i+BLK], dl[i:i+BLK]
            pad = BLK - len(a)
            if pad:
                a = np.concatenate([a, np.zeros(pad, np.int64)])
                b = np.concatenate([b, np.full(pad, 255, np.int64)])
            out.append((a, b))
        return out

    ch = {1: {}, 2: {}}
    for c in range(cfg.CORES):
        m_c = core == c
        s_c, dl_c, bi_c = src[m_c], dst_local[m_c], blk_in_core[m_c]
        rot = (s_c - cfg.SLICE * c) % cfg.NP
        for layer, ids in ((1, rot), (2, s_c)):
            lo = ids < cfg.HALF
            for i in range(cfg.BPC):
                m_b = bi_c == i
                for grp in range(2):
                    m = m_b & (lo if grp == 0 else ~lo)
                    iv = ids[m] - (0 if grp == 0 else cfg.HALF)
                    ch[layer][(c, i, grp)] = chunkify(iv, dl_c[m])

    counts = {}
    for layer in (1, 2):
        nmax = np.zeros((cfg.BPC, 2), np.int64)
        for (c, i, g), lst in ch[layer].items():
            nmax[i, g] = max(nmax[i, g], len(lst))
        counts[layer] = nmax

    per_core = []
    for c in range(cfg.CORES):
        data = {}
        for layer in (1, 2):
            nmax = counts[layer]
            all_chunks = []
            for i in range(cfg.BPC):
                for g in range(2):
                    lst = ch[layer][(c, i, g)]
                    for k in range(int(nmax[i, g])):
                        if k < len(lst):
                            iv, dl = lst[k]
                        else:
                            iv = np.zeros(BLK, np.int64)
                            dl = np.full(BLK, 255, np.int64)
                        all_chunks.append((g, iv, dl))
            totch = len(all_chunks)
            stream = [[], []]
            for g, iv, dl in all_chunks:
                stream[g].append(iv)
            for g in range(2):
                s = stream[g]
                while len(s) % GG:
                    s.append(np.zeros(BLK, np.int64))
                ng = max(1, len(s) // GG)
                arr = np.zeros((128, ng * 64), np.int16)
                for gi in range(len(s) // GG):
                    arr[:, gi*64:(gi+1)*64] = _wrap16(
                        np.concatenate(s[gi*GG:(gi+1)*GG]))
                data[('idxlo' if g == 0 else 'idxhi') + str(layer)] = arr
            metacol = np.zeros((128, totch), np.float32)
            metarow = np.zeros((1, totch * BLK), np.float32)
            for j, (g, iv, dl) in enumerate(all_chunks):
                metacol[:, j] = dl
                metarow[0, j*BLK:(j+1)*BLK] = dl
            import ml_dtypes
            data[f'metacol{layer}'] = metacol
            data[f'metarow{layer}'] = metarow.astype(ml_dtypes.bfloat16)
        per_core.append(data)
    return counts, per_core


def host_weights(inputs, cfg):
    """Weight/constant staging (reordering + transposes only, no math)."""
    W1 = np.asarray(inputs['W1'], np.float32)
    a_src1 = np.asarray(inputs['a_src1'], np.float32)
    a_dst1 = np.asarray(inputs['a_dst1'], np.float32)
    b1 = np.asarray(inputs['b1'], np.float32)
    W2 = np.asarray(inputs['W2'], np.float32)
    a_src2 = np.asarray(inputs['a_src2'], np.float32)
    a_dst2 = np.asarray(inputs['a_dst2'], np.float32)
    b2 = np.asarray(inputs['b2'], np.float32)

    H, C, D1 = cfg.HEADS, cfg.CH, cfg.D1
    perm = np.empty(D1, np.int64)
    for h in range(H):
        for c_ in range(C):
            perm[c_*H + h] = h*C + c_
    consts = {
        'W1cm': W1[:, perm].copy(), 'W1T': W1.T.copy(),
        'a_src1': a_src1, 'a_dst1': a_dst1, 'b1cm': b1[perm][None, :].copy(),
        'W2p': W2[perm, :].copy(), 'W2pT': W2[perm, :].T.copy(),
        'a_src2': a_src2, 'a_dst2': a_dst2, 'b2': b2[None, :].copy(),
        'iota_row': np.broadcast_to(np.arange(128), (128, 128))
                      .astype(__import__('ml_dtypes').bfloat16).copy(),
        'iota_col': np.arange(128, dtype=np.float32)[:, None].copy(),
        'ones_row': np.ones((1, 128), np.float32),
    }
    x = np.asarray(inputs['x'], np.float32)
    xpad = np.zeros((cfg.NP, cfg.F_IN), np.float32)
    xpad[:cfg.N] = x
    xT_rots = [np.roll(xpad, -cfg.SLICE * c, axis=0).T.copy()
               for c in range(cfg.CORES)]
    return consts, xT_rots


# ---------------------------------------------------------------- device side

def build_gat(counts, cfg):
    nc = bacc.Bacc()
    H, C, D1, CLS, F_IN = cfg.HEADS, cfg.CH, cfg.D1, cfg.CLS, cfg.F_IN
    T1, T2 = cfg.TAB1_W, cfg.TAB2_W
    E1 = D1 + 2 * H          # written z1 table cols (z | asrc | adst)
    FH = F_IN // 128

    def n_stream(layer, g):
        return max(1, -(-int(counts[layer][:, g].sum()) // GG))

    GLO1, GHI1 = n_stream(1, 0), n_stream(1, 1)
    GLO2, GHI2 = n_stream(2, 0), n_stream(2, 1)
    TOT1, TOT2 = int(counts[1].sum()), int(counts[2].sum())

    inp = {}
    for name, shape, dt in [
        ('xT_rot', [F_IN, cfg.NP], F32),
        ('W1cm', [F_IN, D1], F32), ('W1T', [D1, F_IN], F32),
        ('a_src1', [H, C], F32), ('a_dst1', [H, C], F32),
        ('b1cm', [1, D1], F32),
        ('W2p', [D1, CLS], F32), ('W2pT', [CLS, D1], F32),
        ('a_src2', [1, CLS], F32), ('a_dst2', [1, CLS], F32),
        ('b2', [1, CLS], F32),
        ('iota_row', [128, 128], BF16), ('iota_col', [128, 1], F32),
        ('ones_row', [1, 128], F32),
        ('idxlo1', [128, GLO1 * 64], I16), ('idxhi1', [128, GHI1 * 64], I16),
        ('idxlo2', [128, GLO2 * 64], I16), ('idxhi2', [128, GHI2 * 64], I16),
        ('metacol1', [128, TOT1], F32), ('metarow1', [1, TOT1 * BLK], BF16),
        ('metacol2', [128, TOT2], F32), ('metarow2', [1, TOT2 * BLK], BF16),
    ]:
        inp[name] = nc.declare_dram_parameter(name, shape, dt, isOutput=False)

    out_d = nc.declare_dram_parameter('out', [cfg.SLICE, CLS], F32, isOutput=True)

    z1tab = nc.dram_tensor('z1tab', [cfg.NP, T1], BF16)
    h1loc = nc.dram_tensor('h1loc', [cfg.SLICE, D1], F32)
    z2slice = nc.dram_tensor('z2slice', [cfg.SLICE, T2], BF16)
    z2cat = nc.dram_tensor('z2cat', [cfg.NP, T2], BF16)

    with tile.TileContext(nc) as tc, ExitStack() as ctx:
        sb = ctx.enter_context(tc.tile_pool(name='sb', bufs=1))
        sbw = ctx.enter_context(tc.tile_pool(name='sbw', bufs=2))

        nc.gpsimd.load_library(library_config.mlp)

        ident = sb.tile([128, 128], F32)
        make_identity(nc, ident[:])
        ident_bf = sb.tile([128, 128], BF16)
        nc.vector.tensor_copy(out=ident_bf[:], in_=ident[:])
        iota_r = sb.tile([128, 128], BF16)
        nc.sync.dma_start(out=iota_r[:], in_=inp['iota_row'][:, :])
        iota_c = sb.tile([128, 1], F32)
        nc.sync.dma_start(out=iota_c[:], in_=inp['iota_col'][:, :])
        ones_r = sb.tile([1, 128], F32)
        nc.sync.dma_start(out=ones_r[:], in_=inp['ones_row'][:, :])
        ones_bf = sb.tile([1, 128], BF16)
        nc.vector.memset(ones_bf[:], 1.0)

        W1e = [sb.tile([128, T1], BF16, tag=f'w1e{_i}', name=f'W1e{_i}') for _i in range(FH)]
        W2e = [sb.tile([128, T2], BF16, tag=f'w2e{_i}', name=f'W2e{_i}') for _i in range(FH)]
        b1b = sb.tile([128, D1], F32)
        b2b = sb.tile([128, CLS], F32)

        with tc.tile_pool(name='p0sb', bufs=1) as p0sb, \
             tc.tile_pool(name='p0ps', bufs=1, space='PSUM') as p0ps:
            for fh in range(FH):
                nc.vector.memset(W1e[fh][:], 0.0)
                nc.vector.memset(W2e[fh][:], 0.0)
            # ---- W1_ext = [W1cm | W1@A_src | W1@A_dst]
            a1 = p0sb.tile([H, 2 * C], F32)
            nc.sync.dma_start(out=a1[:, 0:C], in_=inp['a_src1'][:, :])
            nc.sync.dma_start(out=a1[:, C:2*C], in_=inp['a_dst1'][:, :])
            a1T_ps = p0ps.tile([128, 128], F32, space='PSUM', tag='t')
            nc.tensor.transpose(out=a1T_ps[0:2*C, 0:H], in_=a1[:], identity=ident[0:H, 0:H])
            a1T = p0sb.tile([2 * C, H], F32)
            nc.vector.tensor_copy(out=a1T[:], in_=a1T_ps[0:2*C, 0:H])
            A_bd = p0sb.tile([128, FH, 2 * H], F32)
            nc.vector.memset(A_bd[:], 0.0)
            for h in range(H):
                half, off = divmod(h * C, 128)
                nc.vector.tensor_copy(out=A_bd[off:off+C, half, h:h+1],
                                      in_=a1T[0:C, h:h+1])
                nc.vector.tensor_copy(out=A_bd[off:off+C, half, H+h:H+h+1],
                                      in_=a1T[C:2*C, h:h+1])
            w1t_sb = [p0sb.tile([128, F_IN], F32, tag=f'w1t{_i}', name=f'w1t{_i}') for _i in range(FH)]
            for cc in range(FH):
                nc.sync.dma_start(out=w1t_sb[cc][:],
                                  in_=inp['W1T'][cc*128:(cc+1)*128, :])
            w1a_ps = p0ps.tile([128, FH, 2 * H], F32, space='PSUM', tag='a')
            for fh in range(FH):
                for cc in range(FH):
                    nc.tensor.matmul(out=w1a_ps[:, fh, :],
                                     lhsT=w1t_sb[cc][:, fh*128:(fh+1)*128],
                                     rhs=A_bd[:, cc, :],
                                     start=(cc == 0), stop=(cc == FH - 1))
            for fh in range(FH):
                w1cm_t = p0sb.tile([128, D1], F32, tag='w1cm')
                nc.sync.dma_start(out=w1cm_t[:],
                                  in_=inp['W1cm'][fh*128:(fh+1)*128, :])
                nc.vector.tensor_copy(out=W1e[fh][:, 0:D1], in_=w1cm_t[:])
                nc.vector.tensor_copy(out=W1e[fh][:, D1:D1+2*H],
                                      in_=w1a_ps[:, fh, :])

            # ---- W2_ext = [W2p | W2p@a_src2^T | W2p@a_dst2^T]
            a2 = p0sb.tile([2, CLS], F32)
            nc.sync.dma_start(out=a2[0:1, :], in_=inp['a_src2'][:, :])
            nc.sync.dma_start(out=a2[1:2, :], in_=inp['a_dst2'][:, :])
            a2T_ps = p0ps.tile([128, 128], F32, space='PSUM', tag='t')
            nc.tensor.transpose(out=a2T_ps[0:CLS, 0:2], in_=a2[:], identity=ident[0:2, 0:2])
            a2T = p0sb.tile([CLS, 2], F32)
            nc.vector.tensor_copy(out=a2T[:], in_=a2T_ps[0:CLS, 0:2])
            w2t_sb = p0sb.tile([CLS, D1], F32)
            nc.sync.dma_start(out=w2t_sb[:], in_=inp['W2pT'][:, :])
            w2a_ps = p0ps.tile([128, FH, 2], F32, space='PSUM', tag='a')
            for fh in range(FH):
                nc.tensor.matmul(out=w2a_ps[:, fh, :],
                                 lhsT=w2t_sb[:, fh*128:(fh+1)*128],
                                 rhs=a2T[:], start=True, stop=True)
            for fh in range(FH):
                w2p_t = p0sb.tile([128, CLS], F32, tag='w2p')
                nc.sync.dma_start(out=w2p_t[:],
                                  in_=inp['W2p'][fh*128:(fh+1)*128, :])
                nc.vector.tensor_copy(out=W2e[fh][:, 0:CLS], in_=w2p_t[:])
                nc.vector.tensor_copy(out=W2e[fh][:, CLS:CLS+2],
                                      in_=w2a_ps[:, fh, :])

            # ---- bias broadcast tiles
            b1_sb = p0sb.tile([1, D1], F32)
            nc.sync.dma_start(out=b1_sb[:], in_=inp['b1cm'][:, :])
            b1b_ps = p0ps.tile([128, D1], F32, space='PSUM', tag='b')
            nc.tensor.matmul(out=b1b_ps[:], lhsT=ones_r[:], rhs=b1_sb[:],
                             start=True, stop=True)
            nc.vector.tensor_copy(out=b1b[:], in_=b1b_ps[:])
            b2_sb = p0sb.tile([1, CLS], F32)
            nc.sync.dma_start(out=b2_sb[:], in_=inp['b2'][:, :])
            b2b_ps = p0ps.tile([128, CLS], F32, space='PSUM', tag='b')
            nc.tensor.matmul(out=b2b_ps[:], lhsT=ones_r[:], rhs=b2_sb[:],
                             start=True, stop=True)
            nc.vector.tensor_copy(out=b2b[:], in_=b2b_ps[:])

        # ---- P1: z1 table build (rotated order)
        E1W = E1   # written cols (z | asrc | adst)
        with tc.tile_pool(name='p1sb', bufs=3) as p1sb, \
             tc.tile_pool(name='p1ps', bufs=2, space='PSUM') as p1ps:
            for t in range(cfg.NP // 128):
                zps = p1ps.tile([128, E1W], F32, space='PSUM')
                for fh in range(FH):
                    xt = p1sb.tile([128, 128], F32, tag='xt')
                    nc.sync.dma_start(
                        out=xt[:],
                        in_=inp['xT_rot'][fh*128:(fh+1)*128, t*128:(t+1)*128])
                    xtb = p1sb.tile([128, 128], BF16, tag='xtb')
                    if t % 2 == 0:
                        nc.vector.tensor_copy(out=xtb[:], in_=xt[:])
                    else:
                        nc.scalar.activation(xtb[:], xt[:],
                                             mybir.ActivationFunctionType.Copy)
                    nc.tensor.matmul(out=zps[:], lhsT=xtb[:],
                                     rhs=W1e[fh][:, 0:E1W],
                                     start=(fh == 0), stop=(fh == FH - 1))
                zsb = p1sb.tile([128, E1W], BF16, tag='zsb')
                if t % 2 == 0:
                    nc.scalar.activation(zsb[:], zps[:],
                                         mybir.ActivationFunctionType.Copy)
                else:
                    nc.vector.tensor_copy(out=zsb[:], in_=zps[:])
                nc.sync.dma_start(out=z1tab[t*128:(t+1)*128, 0:E1W], in_=zsb[:])

        # ---- edge phase (shared between layers)
        def edge_phase(layer, tab_lo, tab_hi, tabw, zcols, heads,
                       adst_ap, finalize):
            nmax = counts[layer]
            metarow_d = inp[f'metarow{layer}']
            tot = int(nmax.sum())

            with tc.tile_pool(name=f'ep{layer}', bufs=1) as ep, \
                 tc.tile_pool(name=f'em{layer}', bufs=3) as em, \
                 tc.tile_pool(name=f'eg{layer}', bufs=3) as eg, \
                 tc.tile_pool(name=f'ew{layer}', bufs=3) as ew, \
                 tc.tile_pool(name=f'el{layer}', bufs=2, space='PSUM') as el, \
                 tc.tile_pool(name=f'ea{layer}', bufs=2, space='PSUM') as ea:

                idx_lo = ep.tile([128, inp[f'idxlo{layer}'].shape[1]], I16)
                nc.sync.dma_start(out=idx_lo[:], in_=inp[f'idxlo{layer}'][:, :])
                idx_hi = ep.tile([128, inp[f'idxhi{layer}'].shape[1]], I16)
                nc.sync.dma_start(out=idx_hi[:], in_=inp[f'idxhi{layer}'][:, :])
                metacol = ep.tile([128, tot], F32)
                nc.sync.dma_start(out=metacol[:], in_=inp[f'metacol{layer}'][:, :])

                gtiles = [{}, {}]
                stream_pos = [0, 0]
                mtiles = {}
                chunk_idx = 0
                for i in range(cfg.BPC):
                    nblk = int(nmax[i, 0] + nmax[i, 1])
                    if nblk == 0:
                        continue
                    adst_blk = ew.tile([128, heads], BF16, tag='adst')
                    nc.sync.dma_start(out=adst_blk[:], in_=adst_ap(i))
                    accps = ea.tile([128, zcols + heads], F32, space='PSUM',
                                    tag='acc')
                    done = 0
                    for g in range(2):
                        for _k in range(int(nmax[i, g])):
                            pos = stream_pos[g]
                            stream_pos[g] += 1
                            gi, j = divmod(pos, GG)
                            if gi not in gtiles[g]:
                                gt = eg.tile([128, GG, tabw], BF16, tag=f'g{g}')
                                nc.gpsimd.dma_gather(
                                    out_ap=gt[:],
                                    in_ap=tab_lo if g == 0 else tab_hi,
                                    idxs_ap=(idx_lo if g == 0 else idx_hi)[
                                        :, gi*64:(gi+1)*64],
                                    num_idxs=GG * BLK, num_idxs_reg=GG * BLK,
                                    elem_size=tabw)
                                gtiles[g][gi] = gt
                                for old in [k_ for k_ in gtiles[g]
                                            if k_ < gi - 2]:
                                    del gtiles[g][old]
                            gt = gtiles[g][gi]

                            mg, mj = divmod(chunk_idx, MG)
                            if mg not in mtiles:
                                lo = mg * MG * BLK
                                hi = min(tot * BLK, lo + MG * BLK)
                                mrow = em.tile([1, MG * BLK], BF16, tag='mrow')
                                nc.sync.dma_start(out=mrow[0:1, 0:hi-lo],
                                                  in_=metarow_d[0:1, lo:hi])
                                mb_ps = el.tile([128, MG * BLK], F32,
                                                space='PSUM', tag='mb')
                                nc.tensor.matmul(out=mb_ps[:, 0:hi-lo],
                                                 lhsT=ones_bf[:],
                                                 rhs=mrow[0:1, 0:hi-lo],
                                                 start=True, stop=True)
                                mtiles[mg] = mb_ps
                                for old in [k_ for k_ in mtiles
                                            if k_ < mg - 1]:
                                    del mtiles[old]
                            mb_ps = mtiles[mg]

                            onehot = ew.tile([128, 128], BF16, tag='oh')
                            nc.vector.tensor_scalar(
                                out=onehot[:], in0=iota_r[:],
                                scalar1=metacol[:, chunk_idx:chunk_idx+1],
                                scalar2=None, op0=mybir.AluOpType.is_equal)
                            onehotT = ew.tile([128, 128], BF16, tag='ohT')
                            nc.vector.tensor_scalar(
                                out=onehotT[:],
                                in0=mb_ps[:, mj*BLK:(mj+1)*BLK],
                                scalar1=iota_c[:, 0:1],
                                scalar2=None, op0=mybir.AluOpType.is_equal)

                            lg_ps = el.tile([128, heads], F32, space='PSUM',
                                            tag='lg')
                            nc.tensor.matmul(out=lg_ps[:], lhsT=onehotT[:],
                                             rhs=adst_blk[:],
                                             start=True, stop=False)
                            nc.tensor.matmul(out=lg_ps[:], lhsT=ident_bf[:],
                                             rhs=gt[:, j, zcols:zcols+heads],
                                             start=False, stop=True)
                            lg02 = ew.tile([128, heads], F32, tag='lg02')
                            nc.vector.tensor_scalar(
                                out=lg02[:], in0=lg_ps[:], scalar1=NEG,
                                scalar2=None, op0=mybir.AluOpType.mult)
                            lrl = ew.tile([128, heads], F32, tag='lrl')
                            nc.vector.tensor_tensor(
                                out=lrl[:], in0=lg_ps[:], in1=lg02[:],
                                op=mybir.AluOpType.max)
                            w_t = ew.tile([128, heads], BF16, tag='wt')
                            nc.scalar.activation(w_t[:], lrl[:],
                                                 mybir.ActivationFunctionType.Exp)

                            smsg = ew.tile([128, zcols], BF16, tag='smsg')
                            nc.vector.tensor_tensor(
                                out=smsg[:], in0=gt[:, j, 0:zcols],
                                in1=w_t[:, None, :].to_broadcast(
                                    [128, zcols // heads, heads]),
                                op=mybir.AluOpType.mult)

                            nc.tensor.matmul(out=accps[:, 0:zcols],
                                             lhsT=onehot[:], rhs=smsg[:],
                                             start=(done == 0), stop=False)
                            nc.tensor.matmul(out=accps[:, zcols:zcols+heads],
                                             lhsT=onehot[:], rhs=w_t[:],
                                             start=False,
                                             stop=(done == nblk - 1))
                            done += 1
                            chunk_idx += 1
                    finalize(i, accps)

        # ---- L1 finalize: normalize + bias + elu -> h1loc
        def fin1(i, accps):
            den = sbw.tile([128, H], F32, tag='den')
            nc.vector.tensor_scalar(out=den[:], in0=accps[:, D1:D1+H],
                                    scalar1=1e-30, scalar2=None,
                                    op0=mybir.AluOpType.max)
            rec = sbw.tile([128, H], F32, tag='rec')
            nc.vector.reciprocal(out=rec[:], in_=den[:])
            h1t = sbw.tile([128, D1], F32, tag='h1t')
            nc.vector.tensor_tensor(
                out=h1t[:], in0=accps[:, 0:D1],
                in1=rec[:, None, :].to_broadcast([128, D1 // H, H]),
                op=mybir.AluOpType.mult)
            h1c = sbw.tile([128, D1], F32, tag='h1c')
            nc.vector.tensor_tensor(out=h1c[:], in0=h1t[:], in1=b1b[:],
                                    op=mybir.AluOpType.add)
            # elu(x) = exp(min(x,0)) - 1 + max(x,0)
            m0 = sbw.tile([128, D1], F32, tag='m0')
            nc.vector.tensor_scalar(out=m0[:], in0=h1c[:], scalar1=0.0,
                                    scalar2=None, op0=mybir.AluOpType.min)
            ex = sbw.tile([128, D1], F32, tag='ex')
            nc.scalar.activation(ex[:], m0[:], mybir.ActivationFunctionType.Exp)
            rl = sbw.tile([128, D1], F32, tag='rl')
            nc.vector.tensor_scalar(out=rl[:], in0=h1c[:], scalar1=0.0,
                                    scalar2=None, op0=mybir.AluOpType.max)
            h1f = sbw.tile([128, D1], F32, tag='h1f')
            nc.vector.tensor_tensor(out=h1f[:], in0=ex[:], in1=rl[:],
                                    op=mybir.AluOpType.add)
            h1o = sbw.tile([128, D1], F32, tag='h1o')
            nc.vector.tensor_scalar(out=h1o[:], in0=h1f[:], scalar1=-1.0,
                                    scalar2=None, op0=mybir.AluOpType.add)
            nc.sync.dma_start(out=h1loc[i*128:(i+1)*128, :], in_=h1o[:])

        edge_phase(1, z1tab[0:cfg.HALF, :], z1tab[cfg.HALF:, :], T1, D1, H,
                   lambda i: z1tab[i*128:(i+1)*128, D1+H:D1+2*H], fin1)

        # ---- P3: z2slice = [h1 @ W2p | asrc2 | adst2]
        E2 = CLS + 2
        with tc.tile_pool(name='p3sb', bufs=3) as p3sb, \
             tc.tile_pool(name='p3ps', bufs=2, space='PSUM') as p3ps:
            for t in range(cfg.BPC):
                h1tile = p3sb.tile([128, D1], F32, tag='h1')
                nc.sync.dma_start(out=h1tile[:], in_=h1loc[t*128:(t+1)*128, :])
                zps = p3ps.tile([128, E2], F32, space='PSUM', tag='z')
                for fh in range(FH):
                    tp = p3ps.tile([128, 128], F32, space='PSUM', tag='tp')
                    nc.tensor.transpose(out=tp[:],
                                        in_=h1tile[:, fh*128:(fh+1)*128],
                                        identity=ident[:])
                    h1T = p3sb.tile([128, 128], BF16, tag='h1T')
                    if fh % 2 == 0:
                        nc.vector.tensor_copy(out=h1T[:], in_=tp[:])
                    else:
                        nc.scalar.activation(h1T[:], tp[:],
                                             mybir.ActivationFunctionType.Copy)
                    nc.tensor.matmul(out=zps[:], lhsT=h1T[:],
                                     rhs=W2e[fh][:, 0:E2],
                                     start=(fh == 0), stop=(fh == FH - 1))
                zsb = p3sb.tile([128, E2], BF16, tag='zsb')
                nc.vector.tensor_copy(out=zsb[:], in_=zps[:])
                nc.sync.dma_start(out=z2slice[t*128:(t+1)*128, 0:E2], in_=zsb[:])

        # ---- P4: AllGather z2slice -> z2cat
        nc.gpsimd.collective_compute(
            'AllGather', mybir.AluOpType.bypass,
            replica_groups=[list(range(cfg.CORES))],
            ins=[z2slice.ap().opt()],
            outs=[z2cat.ap().opt()])

        # ---- L2 finalize: normalize + bias -> out
        def fin2(i, accps):
            den = sbw.tile([128, 1], F32, tag='den2')
            nc.vector.tensor_scalar(out=den[:], in0=accps[:, CLS:CLS+1],
                                    scalar1=1e-30, scalar2=None,
                                    op0=mybir.AluOpType.max)
            rec2 = sbw.tile([128, 1], F32, tag='rec2')
            nc.vector.reciprocal(out=rec2[:], in_=den[:])
            o1 = sbw.tile([128, CLS], F32, tag='o1')
            nc.vector.tensor_scalar(out=o1[:], in0=accps[:, 0:CLS],
                                    scalar1=rec2[:, 0:1], scalar2=None,
                                    op0=mybir.AluOpType.mult)
            o2 = sbw.tile([128, CLS], F32, tag='o2')
            nc.vector.tensor_tensor(out=o2[:], in0=o1[:], in1=b2b[:],
                                    op=mybir.AluOpType.add)
            nc.sync.dma_start(out=out_d[i*128:(i+1)*128, :], in_=o2[:])

        edge_phase(2, z2cat[0:cfg.HALF, :], z2cat[cfg.HALF:, :], T2, CLS, 1,
                   lambda i: z2slice[i*128:(i+1)*128, CLS+1:CLS+2], fin2)

    return nc


# ---------------------------------------------------------------- entry point

def run(inputs, cfg, sim=False):
    counts, per_core = host_prep(inputs['edge_index'], cfg)
    consts, xT_rots = host_weights(inputs, cfg)
    nc = build_gat(counts, cfg)
    in_maps = []
    for c in range(cfg.CORES):
        m = dict(consts)
        m['xT_rot'] = xT_rots[c]
        m.update(per_core[c])
        in_maps.append(m)
    if not sim:
        nc.compile()
    if sim:
        from concourse import bass_interp
        ms = bass_interp.MultiCoreSim(nc, cfg.CORES,
                                      num_workers=min(8, cfg.CORES))
        for c in range(cfg.CORES):
            for k, v in in_maps[c].items():
                ms.cores[c].tensor(k)[:] = v
        ms.simulate()
        outs = [np.array(ms.cores[c].mem_tensor('out')).reshape(cfg.SLICE, cfg.CLS)
                for c in range(cfg.CORES)]
    else:
        res = run_bass_kernel_spmd(nc, in_maps, core_ids=list(range(cfg.CORES)))
        outs = [np.asarray(res.results[c]['out']).reshape(cfg.SLICE, cfg.CLS)
                for c in range(cfg.CORES)]
    full = np.concatenate(outs, 0)
    return full[:cfg.N].astype(np.float32)


def kernel(**inputs):
    return run(inputs, FULL, sim=False)



# revision 22
# speedup vs baseline: 1.9157x; 1.2119x over previous
"""Trainium2 Bass kernel for 2-layer GAT (nn_GAT_84146999263862).

Strategy (8 NeuronCores, SPMD):
  - Nodes padded to NP=50176 = 8*49*128; core c owns node slice [c*6272,(c+1)*6272).
  - Edges (plus self-loops) are assigned to cores by dst slice, grouped per
    128-node dst block, packed into 128-edge chunks (dummy pad edges get
    dst_local=255 so their one-hot column is all-zero).
  - Per-edge rows are fetched with gpsimd.dma_gather (int16 indices => tables
    split lo/hi at row 32768; chunks grouped by src half; per-core counts are
    padded to a shared static graph).
  - Per chunk: is_equal one-hot matrices map edges<->dst slots; PE matmuls do
    adst expansion and segment reduction (messages + softmax denominators
    accumulate in PSUM per 128-node block). Softmax skips max-subtraction
    (logits are O(1); mathematically identical).
  - Each core builds its z1 table locally in ROTATED node order (own nodes at
    rows 0..SLICE) so adst/h1 addresses are static; gather indices compensate.
  - One small AllGather shares z2_ext between layers.

Host does integer-only preprocessing (sorting/packing/index maps and pure data
movement like transposes); all floating-point math runs on device.
"""
import sys
import numpy as np

sys.path.insert(0, '/opt/trn_rl_repo')

from contextlib import ExitStack
from concourse import bass, bacc, mybir, tile, library_config
from concourse.bass_utils import run_bass_kernel_spmd
from concourse.masks import make_identity
from concourse.tile import ScopedClock

# This walrus build rejects multi-wait TPB_CTRL instructions; split the Tile
# tail-drain's semaphore waits across single-wait drains.
_MAXW = 1


def _patched_drain_and_barrier(self, tick_clock, wait_clock):
    drain = self.nc.sync.drain()
    wait_clock.add_sem_waits(drain.ins,
                             ScopedClock({None: tick_clock.global_clock}))
    si = drain.ins.sync_info
    waits = list(si.on_wait)
    if len(waits) > _MAXW:
        si.on_wait = waits[:_MAXW]
        for k in range(_MAXW, len(waits), _MAXW):
            extra = self.nc.sync.drain()
            extra.ins.sync_info = mybir.SyncInfo(on_wait=waits[k:k+_MAXW],
                                                 on_update=[])
    self.nc.all_engine_barrier()
    popped = self.nc._tile_sem_poison_stack.pop()
    assert popped is self._sem_poison
    self.nc.clear_and_free_semaphores(list(self.sems.allocated().values()))
    self.nc.all_engine_barrier()


tile.TileContext._drain_and_barrier = _patched_drain_and_barrier

F32 = mybir.dt.float32
BF16 = mybir.dt.bfloat16
I16 = mybir.dt.int16
NEG = 0.2
BLK = 128
GG = 8                     # chunks per dma_gather instruction (1024 indices)
MG = 4                     # chunks per metarow-broadcast matmul (512 cols)


class Cfg:
    def __init__(self, N=50000, cores=8, bpc=49, half=32768,
                 f_in=256, heads=8, ch=32, cls_=32):
        self.N = N
        self.CORES = cores
        self.BPC = bpc
        self.SLICE = bpc * BLK
        self.NP = cores * self.SLICE
        self.HALF = half
        self.F_IN = f_in
        self.HEADS = heads
        self.CH = ch
        self.D1 = heads * ch
        self.CLS = cls_
        # bf16 tables: row byte size must be a multiple of 256 (dma_gather)
        self.TAB1_W = 384                  # row: z(256)|asrc(8)|adst(8)|pad
        self.TAB2_W = 128                  # z2(32)|asrc2|adst2|pad
        assert self.TAB1_W * 2 % 256 == 0 and self.TAB2_W * 2 % 256 == 0
        assert self.HALF % BLK == 0 and self.HALF < 32768 + 1
        assert self.NP - self.HALF <= 32767


FULL = Cfg()


# ---------------------------------------------------------------- host side

def _wrap16(vals):
    """[1024] ints -> [128, 64] int16 (wrapped 16 partitions, replicated x8)."""
    v = np.asarray(vals, np.int64).reshape(64, 16)
    arr = np.zeros((128, 64), np.int16)
    arr[:16, :] = v.T
    for r in range(1, 8):
        arr[r*16:(r+1)*16] = arr[:16]
    return arr


def host_prep(edge_index, cfg):
    """Integer-only preprocessing. Returns (counts, per_core_arrays)."""
    src = np.asarray(edge_index[0], np.int64)
    dst = np.asarray(edge_index[1], np.int64)
    loops = np.arange(cfg.N, dtype=np.int64)
    src = np.concatenate([src, loops])
    dst = np.concatenate([dst, loops])

    core = dst // cfg.SLICE
    blk_in_core = (dst % cfg.SLICE) // BLK
    dst_local = dst % BLK

    def chunkify(iv, dl):
        out = []
        for i in range(0, len(iv), BLK):
            a, b = iv[i:i+BLK], dl[i:i+BLK]
            pad = BLK - len(a)
            if pad:
                a = np.concatenate([a, np.zeros(pad, np.int64)])
                b = np.concatenate([b, np.full(pad, 255, np.int64)])
            out.append((a, b))
        return out

    ch = {1: {}, 2: {}}
    for c in range(cfg.CORES):
        m_c = core == c
        s_c, dl_c, bi_c = src[m_c], dst_local[m_c], blk_in_core[m_c]
        rot = (s_c - cfg.SLICE * c) % cfg.NP
        for layer, ids in ((1, rot), (2, s_c)):
            lo = ids < cfg.HALF
            for i in range(cfg.BPC):
                m_b = bi_c == i
                for grp in range(2):
                    m = m_b & (lo if grp == 0 else ~lo)
                    iv = ids[m] - (0 if grp == 0 else cfg.HALF)
                    ch[layer][(c, i, grp)] = chunkify(iv, dl_c[m])

    counts = {}
    for layer in (1, 2):
        nmax = np.zeros((cfg.BPC, 2), np.int64)
        for (c, i, g), lst in ch[layer].items():
            nmax[i, g] = max(nmax[i, g], len(lst))
        counts[layer] = nmax

    per_core = []
    for c in range(cfg.CORES):
        data = {}
        for layer in (1, 2):
            nmax = counts[layer]
            all_chunks = []
            for i in range(cfg.BPC):
                for g in range(2):
                    lst = ch[layer][(c, i, g)]
                    for k in range(int(nmax[i, g])):
                        if k < len(lst):
                            iv, dl = lst[k]
                        else:
                            iv = np.zeros(BLK, np.int64)
                            dl = np.full(BLK, 255, np.int64)
                        all_chunks.append((g, iv, dl))
            totch = len(all_chunks)
            stream = [[], []]
            for g, iv, dl in all_chunks:
                stream[g].append(iv)
            for g in range(2):
                s = stream[g]
                while len(s) % GG:
                    s.append(np.zeros(BLK, np.int64))
                ng = max(1, len(s) // GG)
                arr = np.zeros((128, ng * 64), np.int16)
                for gi in range(len(s) // GG):
                    arr[:, gi*64:(gi+1)*64] = _wrap16(
                        np.concatenate(s[gi*GG:(gi+1)*GG]))
                data[('idxlo' if g == 0 else 'idxhi') + str(layer)] = arr
            metacol = np.zeros((128, totch), np.float32)
            metarow = np.zeros((1, totch * BLK), np.float32)
            for j, (g, iv, dl) in enumerate(all_chunks):
                metacol[:, j] = dl
                metarow[0, j*BLK:(j+1)*BLK] = dl
            import ml_dtypes
            data[f'metacol{layer}'] = metacol
            data[f'metarow{layer}'] = metarow.astype(ml_dtypes.bfloat16)
        per_core.append(data)
    return counts, per_core


def host_weights(inputs, cfg):
    """Weight/constant staging (reordering + transposes only, no math)."""
    W1 = np.asarray(inputs['W1'], np.float32)
    a_src1 = np.asarray(inputs['a_src1'], np.float32)
    a_dst1 = np.asarray(inputs['a_dst1'], np.float32)
    b1 = np.asarray(inputs['b1'], np.float32)
    W2 = np.asarray(inputs['W2'], np.float32)
    a_src2 = np.asarray(inputs['a_src2'], np.float32)
    a_dst2 = np.asarray(inputs['a_dst2'], np.float32)
    b2 = np.asarray(inputs['b2'], np.float32)

    H, C, D1 = cfg.HEADS, cfg.CH, cfg.D1
    perm = np.empty(D1, np.int64)
    for h in range(H):
        for c_ in range(C):
            perm[c_*H + h] = h*C + c_
    consts = {
        'W1cm': W1[:, perm].copy(), 'W1T': W1.T.copy(),
        'a_src1': a_src1, 'a_dst1': a_dst1, 'b1cm': b1[perm][None, :].copy(),
        'W2p': W2[perm, :].copy(), 'W2pT': W2[perm, :].T.copy(),
        'a_src2': a_src2, 'a_dst2': a_dst2, 'b2': b2[None, :].copy(),
        'iota_row': np.broadcast_to(np.arange(128), (128, 128))
                      .astype(__import__('ml_dtypes').bfloat16).copy(),
        'iota_col': np.arange(128, dtype=np.float32)[:, None].copy(),
        'ones_row': np.ones((1, 128), np.float32),
    }
    x = np.asarray(inputs['x'], np.float32)
    xpad = np.zeros((cfg.NP, cfg.F_IN), np.float32)
    xpad[:cfg.N] = x
    xT_rots = [np.roll(xpad, -cfg.SLICE * c, axis=0).T.copy()
               for c in range(cfg.CORES)]
    return consts, xT_rots


# ---------------------------------------------------------------- device side

def build_gat(counts, cfg):
    nc = bacc.Bacc()
    H, C, D1, CLS, F_IN = cfg.HEADS, cfg.CH, cfg.D1, cfg.CLS, cfg.F_IN
    T1, T2 = cfg.TAB1_W, cfg.TAB2_W
    E1 = D1 + 2 * H          # written z1 table cols (z | asrc | adst)
    FH = F_IN // 128

    def n_stream(layer, g):
        return max(1, -(-int(counts[layer][:, g].sum()) // GG))

    GLO1, GHI1 = n_stream(1, 0), n_stream(1, 1)
    GLO2, GHI2 = n_stream(2, 0), n_stream(2, 1)
    TOT1, TOT2 = int(counts[1].sum()), int(counts[2].sum())

    inp = {}
    for name, shape, dt in [
        ('xT_rot', [F_IN, cfg.NP], F32),
        ('W1cm', [F_IN, D1], F32), ('W1T', [D1, F_IN], F32),
        ('a_src1', [H, C], F32), ('a_dst1', [H, C], F32),
        ('b1cm', [1, D1], F32),
        ('W2p', [D1, CLS], F32), ('W2pT', [CLS, D1], F32),
        ('a_src2', [1, CLS], F32), ('a_dst2', [1, CLS], F32),
        ('b2', [1, CLS], F32),
        ('iota_row', [128, 128], BF16), ('iota_col', [128, 1], F32),
        ('ones_row', [1, 128], F32),
        ('idxlo1', [128, GLO1 * 64], I16), ('idxhi1', [128, GHI1 * 64], I16),
        ('idxlo2', [128, GLO2 * 64], I16), ('idxhi2', [128, GHI2 * 64], I16),
        ('metacol1', [128, TOT1], F32), ('metarow1', [1, TOT1 * BLK], BF16),
        ('metacol2', [128, TOT2], F32), ('metarow2', [1, TOT2 * BLK], BF16),
    ]:
        inp[name] = nc.declare_dram_parameter(name, shape, dt, isOutput=False)

    out_d = nc.declare_dram_parameter('out', [cfg.SLICE, CLS], F32, isOutput=True)

    z1tab = nc.dram_tensor('z1tab', [cfg.NP, T1], BF16)
    h1loc = nc.dram_tensor('h1loc', [cfg.SLICE, D1], F32)
    z2slice = nc.dram_tensor('z2slice', [cfg.SLICE, T2], BF16)
    z2cat = nc.dram_tensor('z2cat', [cfg.NP, T2], BF16)

    with tile.TileContext(nc) as tc, ExitStack() as ctx:
        sb = ctx.enter_context(tc.tile_pool(name='sb', bufs=1))
        sbw = ctx.enter_context(tc.tile_pool(name='sbw', bufs=2))

        nc.gpsimd.load_library(library_config.mlp)

        ident = sb.tile([128, 128], F32)
        make_identity(nc, ident[:])
        ident_bf = sb.tile([128, 128], BF16)
        nc.vector.tensor_copy(out=ident_bf[:], in_=ident[:])
        iota_r = sb.tile([128, 128], BF16)
        nc.sync.dma_start(out=iota_r[:], in_=inp['iota_row'][:, :])
        iota_c = sb.tile([128, 1], F32)
        nc.sync.dma_start(out=iota_c[:], in_=inp['iota_col'][:, :])
        ones_r = sb.tile([1, 128], F32)
        nc.sync.dma_start(out=ones_r[:], in_=inp['ones_row'][:, :])
        ones_bf = sb.tile([1, 128], BF16)
        nc.vector.memset(ones_bf[:], 1.0)

        W1e = [sb.tile([128, T1], BF16, tag=f'w1e{_i}', name=f'W1e{_i}') for _i in range(FH)]
        W2e = [sb.tile([128, T2], BF16, tag=f'w2e{_i}', name=f'W2e{_i}') for _i in range(FH)]
        b1b = sb.tile([128, D1], F32)
        b2b = sb.tile([128, CLS], F32)

        with tc.tile_pool(name='p0sb', bufs=1) as p0sb, \
             tc.tile_pool(name='p0ps', bufs=1, space='PSUM') as p0ps:
            for fh in range(FH):
                nc.vector.memset(W1e[fh][:], 0.0)
                nc.vector.memset(W2e[fh][:], 0.0)
            # ---- W1_ext = [W1cm | W1@A_src | W1@A_dst]
            a1 = p0sb.tile([H, 2 * C], F32)
            nc.sync.dma_start(out=a1[:, 0:C], in_=inp['a_src1'][:, :])
            nc.sync.dma_start(out=a1[:, C:2*C], in_=inp['a_dst1'][:, :])
            a1T_ps = p0ps.tile([128, 128], F32, space='PSUM', tag='t')
            nc.tensor.transpose(out=a1T_ps[0:2*C, 0:H], in_=a1[:], identity=ident[0:H, 0:H])
            a1T = p0sb.tile([2 * C, H], F32)
            nc.vector.tensor_copy(out=a1T[:], in_=a1T_ps[0:2*C, 0:H])
            A_bd = p0sb.tile([128, FH, 2 * H], F32)
            nc.vector.memset(A_bd[:], 0.0)
            for h in range(H):
                half, off = divmod(h * C, 128)
                nc.vector.tensor_copy(out=A_bd[off:off+C, half, h:h+1],
                                      in_=a1T[0:C, h:h+1])
                nc.vector.tensor_copy(out=A_bd[off:off+C, half, H+h:H+h+1],
                                      in_=a1T[C:2*C, h:h+1])
            w1t_sb = [p0sb.tile([128, F_IN], F32, tag=f'w1t{_i}', name=f'w1t{_i}') for _i in range(FH)]
            for cc in range(FH):
                nc.sync.dma_start(out=w1t_sb[cc][:],
                                  in_=inp['W1T'][cc*128:(cc+1)*128, :])
            w1a_ps = p0ps.tile([128, FH, 2 * H], F32, space='PSUM', tag='a')
            for fh in range(FH):
                for cc in range(FH):
                    nc.tensor.matmul(out=w1a_ps[:, fh, :],
                                     lhsT=w1t_sb[cc][:, fh*128:(fh+1)*128],
                                     rhs=A_bd[:, cc, :],
                                     start=(cc == 0), stop=(cc == FH - 1))
            for fh in range(FH):
                w1cm_t = p0sb.tile([128, D1], F32, tag='w1cm')
                nc.sync.dma_start(out=w1cm_t[:],
                                  in_=inp['W1cm'][fh*128:(fh+1)*128, :])
                nc.vector.tensor_copy(out=W1e[fh][:, 0:D1], in_=w1cm_t[:])
                nc.vector.tensor_copy(out=W1e[fh][:, D1:D1+2*H],
                                      in_=w1a_ps[:, fh, :])

            # ---- W2_ext = [W2p | W2p@a_src2^T | W2p@a_dst2^T]
            a2 = p0sb.tile([2, CLS], F32)
            nc.sync.dma_start(out=a2[0:1, :], in_=inp['a_src2'][:, :])
            nc.sync.dma_start(out=a2[1:2, :], in_=inp['a_dst2'][:, :])
            a2T_ps = p0ps.tile([128, 128], F32, space='PSUM', tag='t')
            nc.tensor.transpose(out=a2T_ps[0:CLS, 0:2], in_=a2[:], identity=ident[0:2, 0:2])
            a2T = p0sb.tile([CLS, 2], F32)
            nc.vector.tensor_copy(out=a2T[:], in_=a2T_ps[0:CLS, 0:2])
            w2t_sb = p0sb.tile([CLS, D1], F32)
            nc.sync.dma_start(out=w2t_sb[:], in_=inp['W2pT'][:, :])
            w2a_ps = p0ps.tile([128, FH, 2], F32, space='PSUM', tag='a')
            for fh in range(FH):
                nc.tensor.matmul(out=w2a_ps[:, fh, :],
                                 lhsT=w2t_sb[:, fh*128:(fh+1)*128],
                                 rhs=a2T[:], start=True, stop=True)
            for fh in range(FH):
                w2p_t = p0sb.tile([128, CLS], F32, tag='w2p')
                nc.sync.dma_start(out=w2p_t[:],
                                  in_=inp['W2p'][fh*128:(fh+1)*128, :])
                nc.vector.tensor_copy(out=W2e[fh][:, 0:CLS], in_=w2p_t[:])
                nc.vector.tensor_copy(out=W2e[fh][:, CLS:CLS+2],
                                      in_=w2a_ps[:, fh, :])

            # ---- bias broadcast tiles
            b1_sb = p0sb.tile([1, D1], F32)
            nc.sync.dma_start(out=b1_sb[:], in_=inp['b1cm'][:, :])
            b1b_ps = p0ps.tile([128, D1], F32, space='PSUM', tag='b')
            nc.tensor.matmul(out=b1b_ps[:], lhsT=ones_r[:], rhs=b1_sb[:],
                             start=True, stop=True)
            nc.vector.tensor_copy(out=b1b[:], in_=b1b_ps[:])
            b2_sb = p0sb.tile([1, CLS], F32)
            nc.sync.dma_start(out=b2_sb[:], in_=inp['b2'][:, :])
            b2b_ps = p0ps.tile([128, CLS], F32, space='PSUM', tag='b')
            nc.tensor.matmul(out=b2b_ps[:], lhsT=ones_r[:], rhs=b2_sb[:],
                             start=True, stop=True)
            nc.vector.tensor_copy(out=b2b[:], in_=b2b_ps[:])

        # ---- P1: z1 table build (rotated order)
        E1W = E1   # written cols (z | asrc | adst)
        T4 = 4      # node tiles per batched x load
        with tc.tile_pool(name='p1sb', bufs=3) as p1sb, \
             tc.tile_pool(name='p1ps', bufs=2, space='PSUM') as p1ps:
            for t4 in range(0, cfg.NP // 128, T4):
                xtb4 = []
                for fh in range(FH):
                    xt = p1sb.tile([128, T4 * 128], F32, tag=f'xt{fh}')
                    nc.sync.dma_start(
                        out=xt[:],
                        in_=inp['xT_rot'][fh*128:(fh+1)*128,
                                          t4*128:(t4+T4)*128])
                    xtb = p1sb.tile([128, T4 * 128], BF16, tag=f'xtb{fh}')
                    nc.vector.tensor_copy(out=xtb[:], in_=xt[:])
                    xtb4.append(xtb)
                for k in range(T4):
                    t = t4 + k
                    zps = p1ps.tile([128, E1W], F32, space='PSUM')
                    for fh in range(FH):
                        nc.tensor.matmul(out=zps[:],
                                         lhsT=xtb4[fh][:, k*128:(k+1)*128],
                                         rhs=W1e[fh][:, 0:E1W],
                                         start=(fh == 0), stop=(fh == FH - 1))
                    zsb = p1sb.tile([128, E1W], BF16, tag='zsb')
                    if t % 2 == 0:
                        nc.scalar.activation(zsb[:], zps[:],
                                             mybir.ActivationFunctionType.Copy)
                    else:
                        nc.vector.tensor_copy(out=zsb[:], in_=zps[:])
                    nc.scalar.dma_start(out=z1tab[t*128:(t+1)*128, 0:E1W],
                                        in_=zsb[:])

        # ---- edge phase (shared between layers)
        def edge_phase(layer, tab_lo, tab_hi, tabw, zcols, heads,
                       adst_ap, finalize):
            nmax = counts[layer]
            metarow_d = inp[f'metarow{layer}']
            tot = int(nmax.sum())

            with tc.tile_pool(name=f'ep{layer}', bufs=1) as ep, \
                 tc.tile_pool(name=f'em{layer}', bufs=3) as em, \
                 tc.tile_pool(name=f'eg{layer}', bufs=3) as eg, \
                 tc.tile_pool(name=f'ew{layer}', bufs=3) as ew, \
                 tc.tile_pool(name=f'el{layer}', bufs=2, space='PSUM') as el, \
                 tc.tile_pool(name=f'ea{layer}', bufs=2, space='PSUM') as ea:

                idx_lo = ep.tile([128, inp[f'idxlo{layer}'].shape[1]], I16)
                nc.sync.dma_start(out=idx_lo[:], in_=inp[f'idxlo{layer}'][:, :])
                idx_hi = ep.tile([128, inp[f'idxhi{layer}'].shape[1]], I16)
                nc.sync.dma_start(out=idx_hi[:], in_=inp[f'idxhi{layer}'][:, :])
                metacol = ep.tile([128, tot], F32)
                nc.sync.dma_start(out=metacol[:], in_=inp[f'metacol{layer}'][:, :])

                gtiles = [{}, {}]
                stream_pos = [0, 0]
                mtiles = {}
                chunk_idx = 0
                for i in range(cfg.BPC):
                    nblk = int(nmax[i, 0] + nmax[i, 1])
                    if nblk == 0:
                        continue
                    adst_blk = ew.tile([128, heads], BF16, tag='adst')
                    nc.sync.dma_start(out=adst_blk[:], in_=adst_ap(i))
                    accps = ea.tile([128, zcols + heads], F32, space='PSUM',
                                    tag='acc')
                    done = 0
                    for g in range(2):
                        for _k in range(int(nmax[i, g])):
                            pos = stream_pos[g]
                            stream_pos[g] += 1
                            gi, j = divmod(pos, GG)
                            if gi not in gtiles[g]:
                                gt = eg.tile([128, GG, tabw], BF16, tag=f'g{g}')
                                nc.gpsimd.dma_gather(
                                    out_ap=gt[:],
                                    in_ap=tab_lo if g == 0 else tab_hi,
                                    idxs_ap=(idx_lo if g == 0 else idx_hi)[
                                        :, gi*64:(gi+1)*64],
                                    num_idxs=GG * BLK, num_idxs_reg=GG * BLK,
                                    elem_size=tabw)
                                gtiles[g][gi] = gt
                                for old in [k_ for k_ in gtiles[g]
                                            if k_ < gi - 2]:
                                    del gtiles[g][old]
                            gt = gtiles[g][gi]

                            mg, mj = divmod(chunk_idx, MG)
                            if mg not in mtiles:
                                lo = mg * MG * BLK
                                hi = min(tot * BLK, lo + MG * BLK)
                                mrow = em.tile([1, MG * BLK], BF16, tag='mrow')
                                nc.sync.dma_start(out=mrow[0:1, 0:hi-lo],
                                                  in_=metarow_d[0:1, lo:hi])
                                mb_ps = el.tile([128, MG * BLK], F32,
                                                space='PSUM', tag='mb')
                                nc.tensor.matmul(out=mb_ps[:, 0:hi-lo],
                                                 lhsT=ones_bf[:],
                                                 rhs=mrow[0:1, 0:hi-lo],
                                                 start=True, stop=True)
                                ohT_b = em.tile([128, MG * BLK], BF16,
                                                tag='ohtb')
                                nc.vector.tensor_scalar(
                                    out=ohT_b[:, 0:hi-lo],
                                    in0=mb_ps[:, 0:hi-lo],
                                    scalar1=iota_c[:, 0:1],
                                    scalar2=None, op0=mybir.AluOpType.is_equal)
                                mtiles[mg] = ohT_b
                                for old in [k_ for k_ in mtiles
                                            if k_ < mg - 1]:
                                    del mtiles[old]
                            ohT_b = mtiles[mg]

                            onehot = ew.tile([128, 128], BF16, tag='oh')
                            nc.vector.tensor_scalar(
                                out=onehot[:], in0=iota_r[:],
                                scalar1=metacol[:, chunk_idx:chunk_idx+1],
                                scalar2=None, op0=mybir.AluOpType.is_equal)

                            lg_ps = el.tile([128, heads], F32, space='PSUM',
                                            tag='lg')
                            nc.tensor.matmul(out=lg_ps[:],
                                             lhsT=ohT_b[:, mj*BLK:(mj+1)*BLK],
                                             rhs=adst_blk[:],
                                             start=True, stop=False)
                            nc.tensor.matmul(out=lg_ps[:], lhsT=ident_bf[:],
                                             rhs=gt[:, j, zcols:zcols+heads],
                                             start=False, stop=True)
                            # exp(lrelu(x)) == max(exp(x), exp(0.2x))
                            e1 = ew.tile([128, heads], F32, tag='e1')
                            nc.scalar.activation(e1[:], lg_ps[:],
                                                 mybir.ActivationFunctionType.Exp)
                            e2 = ew.tile([128, heads], F32, tag='e2')
                            nc.scalar.activation(e2[:], lg_ps[:],
                                                 mybir.ActivationFunctionType.Exp,
                                                 scale=NEG)
                            w_t = ew.tile([128, heads], BF16, tag='wt')
                            nc.vector.tensor_tensor(
                                out=w_t[:], in0=e1[:], in1=e2[:],
                                op=mybir.AluOpType.max)

                            smsg = ew.tile([128, zcols], BF16, tag='smsg')
                            nc.vector.tensor_tensor(
                                out=smsg[:], in0=gt[:, j, 0:zcols],
                                in1=w_t[:, None, :].to_broadcast(
                                    [128, zcols // heads, heads]),
                                op=mybir.AluOpType.mult)

                            nc.tensor.matmul(out=accps[:, 0:zcols],
                                             lhsT=onehot[:], rhs=smsg[:],
                                             start=(done == 0), stop=False)
                            nc.tensor.matmul(out=accps[:, zcols:zcols+heads],
                                             lhsT=onehot[:], rhs=w_t[:],
                                             start=False,
                                             stop=(done == nblk - 1))
                            done += 1
                            chunk_idx += 1
                    finalize(i, accps)

        # ---- L1 finalize: normalize + bias + elu -> h1loc
        def fin1(i, accps):
            den = sbw.tile([128, H], F32, tag='den')
            nc.vector.tensor_scalar(out=den[:], in0=accps[:, D1:D1+H],
                                    scalar1=1e-30, scalar2=None,
                                    op0=mybir.AluOpType.max)
            rec = sbw.tile([128, H], F32, tag='rec')
            nc.vector.reciprocal(out=rec[:], in_=den[:])
            h1t = sbw.tile([128, D1], F32, tag='h1t')
            nc.vector.tensor_tensor(
                out=h1t[:], in0=accps[:, 0:D1],
                in1=rec[:, None, :].to_broadcast([128, D1 // H, H]),
                op=mybir.AluOpType.mult)
            h1c = sbw.tile([128, D1], F32, tag='h1c')
            nc.vector.tensor_tensor(out=h1c[:], in0=h1t[:], in1=b1b[:],
                                    op=mybir.AluOpType.add)
            # elu(x) = exp(min(x,0)) - 1 + max(x,0)
            m0 = sbw.tile([128, D1], F32, tag='m0')
            nc.vector.tensor_scalar(out=m0[:], in0=h1c[:], scalar1=0.0,
                                    scalar2=None, op0=mybir.AluOpType.min)
            ex = sbw.tile([128, D1], F32, tag='ex')
            nc.scalar.activation(ex[:], m0[:], mybir.ActivationFunctionType.Exp)
            rl = sbw.tile([128, D1], F32, tag='rl')
            nc.vector.tensor_scalar(out=rl[:], in0=h1c[:], scalar1=0.0,
                                    scalar2=None, op0=mybir.AluOpType.max)
            h1f = sbw.tile([128, D1], F32, tag='h1f')
            nc.vector.tensor_tensor(out=h1f[:], in0=ex[:], in1=rl[:],
                                    op=mybir.AluOpType.add)
            h1o = sbw.tile([128, D1], F32, tag='h1o')
            nc.vector.tensor_scalar(out=h1o[:], in0=h1f[:], scalar1=-1.0,
                                    scalar2=None, op0=mybir.AluOpType.add)
            nc.sync.dma_start(out=h1loc[i*128:(i+1)*128, :], in_=h1o[:])

        edge_phase(1, z1tab[0:cfg.HALF, :], z1tab[cfg.HALF:, :], T1, D1, H,
                   lambda i: z1tab[i*128:(i+1)*128, D1+H:D1+2*H], fin1)

        # ---- P3: z2slice = [h1 @ W2p | asrc2 | adst2]
        E2 = CLS + 2
        with tc.tile_pool(name='p3sb', bufs=3) as p3sb, \
             tc.tile_pool(name='p3ps', bufs=2, space='PSUM') as p3ps:
            for t in range(cfg.BPC):
                h1tile = p3sb.tile([128, D1], F32, tag='h1')
                nc.sync.dma_start(out=h1tile[:], in_=h1loc[t*128:(t+1)*128, :])
                zps = p3ps.tile([128, E2], F32, space='PSUM', tag='z')
                for fh in range(FH):
                    tp = p3ps.tile([128, 128], F32, space='PSUM', tag='tp')
                    nc.tensor.transpose(out=tp[:],
                                        in_=h1tile[:, fh*128:(fh+1)*128],
                                        identity=ident[:])
                    h1T = p3sb.tile([128, 128], BF16, tag='h1T')
                    if fh % 2 == 0:
                        nc.vector.tensor_copy(out=h1T[:], in_=tp[:])
                    else:
                        nc.scalar.activation(h1T[:], tp[:],
                                             mybir.ActivationFunctionType.Copy)
                    nc.tensor.matmul(out=zps[:], lhsT=h1T[:],
                                     rhs=W2e[fh][:, 0:E2],
                                     start=(fh == 0), stop=(fh == FH - 1))
                zsb = p3sb.tile([128, E2], BF16, tag='zsb')
                nc.vector.tensor_copy(out=zsb[:], in_=zps[:])
                nc.sync.dma_start(out=z2slice[t*128:(t+1)*128, 0:E2], in_=zsb[:])

        # ---- P4: AllGather z2slice -> z2cat
        nc.gpsimd.collective_compute(
            'AllGather', mybir.AluOpType.bypass,
            replica_groups=[list(range(cfg.CORES))],
            ins=[z2slice.ap().opt()],
            outs=[z2cat.ap().opt()])

        # ---- L2 finalize: normalize + bias -> out
        def fin2(i, accps):
            den = sbw.tile([128, 1], F32, tag='den2')
            nc.vector.tensor_scalar(out=den[:], in0=accps[:, CLS:CLS+1],
                                    scalar1=1e-30, scalar2=None,
                                    op0=mybir.AluOpType.max)
            rec2 = sbw.tile([128, 1], F32, tag='rec2')
            nc.vector.reciprocal(out=rec2[:], in_=den[:])
            o1 = sbw.tile([128, CLS], F32, tag='o1')
            nc.vector.tensor_scalar(out=o1[:], in0=accps[:, 0:CLS],
                                    scalar1=rec2[:, 0:1], scalar2=None,
                                    op0=mybir.AluOpType.mult)
            o2 = sbw.tile([128, CLS], F32, tag='o2')
            nc.vector.tensor_tensor(out=o2[:], in0=o1[:], in1=b2b[:],
                                    op=mybir.AluOpType.add)
            nc.sync.dma_start(out=out_d[i*128:(i+1)*128, :], in_=o2[:])

        edge_phase(2, z2cat[0:cfg.HALF, :], z2cat[cfg.HALF:, :], T2, CLS, 1,
                   lambda i: z2slice[i*128:(i+1)*128, CLS+1:CLS+2], fin2)

    return nc


# ---------------------------------------------------------------- entry point

def run(inputs, cfg, sim=False):
    counts, per_core = host_prep(inputs['edge_index'], cfg)
    consts, xT_rots = host_weights(inputs, cfg)
    nc = build_gat(counts, cfg)
    in_maps = []
    for c in range(cfg.CORES):
        m = dict(consts)
        m['xT_rot'] = xT_rots[c]
        m.update(per_core[c])
        in_maps.append(m)
    if not sim:
        nc.compile()
    if sim:
        from concourse import bass_interp
        ms = bass_interp.MultiCoreSim(nc, cfg.CORES,
                                      num_workers=min(8, cfg.CORES))
        for c in range(cfg.CORES):
            for k, v in in_maps[c].items():
                ms.cores[c].tensor(k)[:] = v
        ms.simulate()
        outs = [np.array(ms.cores[c].mem_tensor('out')).reshape(cfg.SLICE, cfg.CLS)
                for c in range(cfg.CORES)]
    else:
        res = run_bass_kernel_spmd(nc, in_maps, core_ids=list(range(cfg.CORES)))
        outs = [np.asarray(res.results[c]['out']).reshape(cfg.SLICE, cfg.CLS)
                for c in range(cfg.CORES)]
    full = np.concatenate(outs, 0)
    return full[:cfg.N].astype(np.float32)


def kernel(**inputs):
    return run(inputs, FULL, sim=False)



# revision 23
# speedup vs baseline: 2.3000x; 1.2006x over previous
"""Trainium2 Bass kernel for 2-layer GAT (nn_GAT_84146999263862).

Strategy (8 NeuronCores, SPMD):
  - Nodes padded to NP=50176 = 8*49*128; core c owns node slice [c*6272,(c+1)*6272).
  - Edges (plus self-loops) are assigned to cores by dst slice, grouped per
    128-node dst block, packed into 128-edge chunks (dummy pad edges get
    dst_local=255 so their one-hot column is all-zero).
  - Per-edge rows are fetched with gpsimd.dma_gather (int16 indices => tables
    split lo/hi at row 32768; chunks grouped by src half; per-core counts are
    padded to a shared static graph).
  - Per chunk: is_equal one-hot matrices map edges<->dst slots; PE matmuls do
    adst expansion and segment reduction (messages + softmax denominators
    accumulate in PSUM per 128-node block). Softmax skips max-subtraction
    (logits are O(1); mathematically identical).
  - Each core builds its z1 table locally in ROTATED node order (own nodes at
    rows 0..SLICE) so adst/h1 addresses are static; gather indices compensate.
  - One small AllGather shares z2_ext between layers.

Host does integer-only preprocessing (sorting/packing/index maps and pure data
movement like transposes); all floating-point math runs on device.
"""
import sys
import numpy as np

sys.path.insert(0, '/opt/trn_rl_repo')

from contextlib import ExitStack
from concourse import bass, bacc, mybir, tile, library_config
from concourse.bass_utils import run_bass_kernel_spmd
from concourse.masks import make_identity
from concourse.tile import ScopedClock

# This walrus build rejects multi-wait TPB_CTRL instructions; split the Tile
# tail-drain's semaphore waits across single-wait drains.
_MAXW = 1


def _patched_drain_and_barrier(self, tick_clock, wait_clock):
    drain = self.nc.sync.drain()
    wait_clock.add_sem_waits(drain.ins,
                             ScopedClock({None: tick_clock.global_clock}))
    si = drain.ins.sync_info
    waits = list(si.on_wait)
    if len(waits) > _MAXW:
        si.on_wait = waits[:_MAXW]
        for k in range(_MAXW, len(waits), _MAXW):
            extra = self.nc.sync.drain()
            extra.ins.sync_info = mybir.SyncInfo(on_wait=waits[k:k+_MAXW],
                                                 on_update=[])
    self.nc.all_engine_barrier()
    popped = self.nc._tile_sem_poison_stack.pop()
    assert popped is self._sem_poison
    self.nc.clear_and_free_semaphores(list(self.sems.allocated().values()))
    self.nc.all_engine_barrier()


tile.TileContext._drain_and_barrier = _patched_drain_and_barrier

F32 = mybir.dt.float32
BF16 = mybir.dt.bfloat16
I16 = mybir.dt.int16
NEG = 0.2
BLK = 128
GG = 8                     # chunks per dma_gather instruction (1024 indices)
MG = 4                     # chunks per metarow-broadcast matmul (512 cols)


class Cfg:
    def __init__(self, N=50000, cores=8, bpc=49, half=32768,
                 f_in=256, heads=8, ch=32, cls_=32):
        self.N = N
        self.CORES = cores
        self.BPC = bpc
        self.SLICE = bpc * BLK
        self.NP = cores * self.SLICE
        self.HALF = half
        self.F_IN = f_in
        self.HEADS = heads
        self.CH = ch
        self.D1 = heads * ch
        self.CLS = cls_
        # bf16 tables: row byte size must be a multiple of 256 (dma_gather)
        self.TAB1_W = 384                  # row: z(256)|asrc(8)|adst(8)|pad
        self.TAB2_W = 128                  # z2(32)|asrc2|adst2|pad
        assert self.TAB1_W * 2 % 256 == 0 and self.TAB2_W * 2 % 256 == 0
        assert self.HALF % BLK == 0 and self.HALF < 32768 + 1
        assert self.NP - self.HALF <= 32767


FULL = Cfg()


# ---------------------------------------------------------------- host side

def _wrap16(vals):
    """[1024] ints -> [128, 64] int16 (wrapped 16 partitions, replicated x8)."""
    v = np.asarray(vals, np.int64).reshape(64, 16)
    arr = np.zeros((128, 64), np.int16)
    arr[:16, :] = v.T
    for r in range(1, 8):
        arr[r*16:(r+1)*16] = arr[:16]
    return arr


def host_prep(edge_index, cfg):
    """Integer-only preprocessing. Returns (counts, per_core_arrays)."""
    src = np.asarray(edge_index[0], np.int64)
    dst = np.asarray(edge_index[1], np.int64)
    loops = np.arange(cfg.N, dtype=np.int64)
    src = np.concatenate([src, loops])
    dst = np.concatenate([dst, loops])

    core = dst // cfg.SLICE
    blk_in_core = (dst % cfg.SLICE) // BLK
    dst_local = dst % BLK

    def chunkify(iv, dl):
        out = []
        for i in range(0, len(iv), BLK):
            a, b = iv[i:i+BLK], dl[i:i+BLK]
            pad = BLK - len(a)
            if pad:
                a = np.concatenate([a, np.zeros(pad, np.int64)])
                b = np.concatenate([b, np.full(pad, 255, np.int64)])
            out.append((a, b))
        return out

    ch = {1: {}, 2: {}}
    for c in range(cfg.CORES):
        m_c = core == c
        s_c, dl_c, bi_c = src[m_c], dst_local[m_c], blk_in_core[m_c]
        rot = (s_c - cfg.SLICE * c) % cfg.NP
        for layer, ids in ((1, rot), (2, s_c)):
            lo = ids < cfg.HALF
            for i in range(cfg.BPC):
                m_b = bi_c == i
                for grp in range(2):
                    m = m_b & (lo if grp == 0 else ~lo)
                    iv = ids[m] - (0 if grp == 0 else cfg.HALF)
                    ch[layer][(c, i, grp)] = chunkify(iv, dl_c[m])

    counts = {}
    for layer in (1, 2):
        nmax = np.zeros((cfg.BPC, 2), np.int64)
        for (c, i, g), lst in ch[layer].items():
            nmax[i, g] = max(nmax[i, g], len(lst))
        counts[layer] = nmax

    per_core = []
    for c in range(cfg.CORES):
        data = {}
        for layer in (1, 2):
            nmax = counts[layer]
            all_chunks = []
            for i in range(cfg.BPC):
                for g in range(2):
                    lst = ch[layer][(c, i, g)]
                    for k in range(int(nmax[i, g])):
                        if k < len(lst):
                            iv, dl = lst[k]
                        else:
                            iv = np.zeros(BLK, np.int64)
                            dl = np.full(BLK, 255, np.int64)
                        all_chunks.append((g, iv, dl))
            totch = len(all_chunks)
            stream = [[], []]
            for g, iv, dl in all_chunks:
                stream[g].append(iv)
            for g in range(2):
                s = stream[g]
                while len(s) % GG:
                    s.append(np.zeros(BLK, np.int64))
                ng = max(1, len(s) // GG)
                arr = np.zeros((128, ng * 64), np.int16)
                for gi in range(len(s) // GG):
                    arr[:, gi*64:(gi+1)*64] = _wrap16(
                        np.concatenate(s[gi*GG:(gi+1)*GG]))
                data[('idxlo' if g == 0 else 'idxhi') + str(layer)] = arr
            metacol = np.zeros((128, totch), np.float32)
            metarow = np.zeros((1, totch * BLK), np.float32)
            for j, (g, iv, dl) in enumerate(all_chunks):
                metacol[:, j] = dl
                metarow[0, j*BLK:(j+1)*BLK] = dl
            import ml_dtypes
            data[f'metacol{layer}'] = metacol.astype(ml_dtypes.bfloat16)
            data[f'metarow{layer}'] = metarow.astype(ml_dtypes.bfloat16)
        per_core.append(data)
    return counts, per_core


def host_weights(inputs, cfg):
    """Weight/constant staging (reordering + transposes only, no math)."""
    W1 = np.asarray(inputs['W1'], np.float32)
    a_src1 = np.asarray(inputs['a_src1'], np.float32)
    a_dst1 = np.asarray(inputs['a_dst1'], np.float32)
    b1 = np.asarray(inputs['b1'], np.float32)
    W2 = np.asarray(inputs['W2'], np.float32)
    a_src2 = np.asarray(inputs['a_src2'], np.float32)
    a_dst2 = np.asarray(inputs['a_dst2'], np.float32)
    b2 = np.asarray(inputs['b2'], np.float32)

    H, C, D1 = cfg.HEADS, cfg.CH, cfg.D1
    perm = np.empty(D1, np.int64)
    for h in range(H):
        for c_ in range(C):
            perm[c_*H + h] = h*C + c_
    consts = {
        'W1cm': W1[:, perm].copy(), 'W1T': W1.T.copy(),
        'a_src1': a_src1, 'a_dst1': a_dst1, 'b1cm': b1[perm][None, :].copy(),
        'W2p': W2[perm, :].copy(), 'W2pT': W2[perm, :].T.copy(),
        'a_src2': a_src2, 'a_dst2': a_dst2, 'b2': b2[None, :].copy(),
        'iota_row': np.broadcast_to(np.arange(128), (128, 128))
                      .astype(__import__('ml_dtypes').bfloat16).copy(),
        'iota_col': np.arange(128, dtype=np.float32)[:, None].copy(),
        'ones_row': np.ones((1, 128), np.float32),
    }
    x = np.asarray(inputs['x'], np.float32)
    xpad = np.zeros((cfg.NP, cfg.F_IN), np.float32)
    xpad[:cfg.N] = x
    xT_rots = [np.roll(xpad, -cfg.SLICE * c, axis=0).T.copy()
               for c in range(cfg.CORES)]
    return consts, xT_rots


# ---------------------------------------------------------------- device side

def build_gat(counts, cfg):
    nc = bacc.Bacc()
    H, C, D1, CLS, F_IN = cfg.HEADS, cfg.CH, cfg.D1, cfg.CLS, cfg.F_IN
    T1, T2 = cfg.TAB1_W, cfg.TAB2_W
    E1 = D1 + 2 * H          # written z1 table cols (z | asrc | adst)
    FH = F_IN // 128

    def n_stream(layer, g):
        return max(1, -(-int(counts[layer][:, g].sum()) // GG))

    GLO1, GHI1 = n_stream(1, 0), n_stream(1, 1)
    GLO2, GHI2 = n_stream(2, 0), n_stream(2, 1)
    TOT1, TOT2 = int(counts[1].sum()), int(counts[2].sum())

    inp = {}
    for name, shape, dt in [
        ('xT_rot', [F_IN, cfg.NP], F32),
        ('W1cm', [F_IN, D1], F32), ('W1T', [D1, F_IN], F32),
        ('a_src1', [H, C], F32), ('a_dst1', [H, C], F32),
        ('b1cm', [1, D1], F32),
        ('W2p', [D1, CLS], F32), ('W2pT', [CLS, D1], F32),
        ('a_src2', [1, CLS], F32), ('a_dst2', [1, CLS], F32),
        ('b2', [1, CLS], F32),
        ('iota_row', [128, 128], BF16), ('iota_col', [128, 1], F32),
        ('ones_row', [1, 128], F32),
        ('idxlo1', [128, GLO1 * 64], I16), ('idxhi1', [128, GHI1 * 64], I16),
        ('idxlo2', [128, GLO2 * 64], I16), ('idxhi2', [128, GHI2 * 64], I16),
        ('metacol1', [128, TOT1], BF16), ('metarow1', [1, TOT1 * BLK], BF16),
        ('metacol2', [128, TOT2], BF16), ('metarow2', [1, TOT2 * BLK], BF16),
    ]:
        inp[name] = nc.declare_dram_parameter(name, shape, dt, isOutput=False)

    out_d = nc.declare_dram_parameter('out', [cfg.SLICE, CLS], F32, isOutput=True)

    z1tab = nc.dram_tensor('z1tab', [cfg.NP, T1], BF16)
    h1loc = nc.dram_tensor('h1loc', [cfg.SLICE, D1], F32)
    z2slice = nc.dram_tensor('z2slice', [cfg.SLICE, T2], BF16)
    z2cat = nc.dram_tensor('z2cat', [cfg.NP, T2], BF16)

    with tile.TileContext(nc) as tc, ExitStack() as ctx:
        sb = ctx.enter_context(tc.tile_pool(name='sb', bufs=1))
        sbw = ctx.enter_context(tc.tile_pool(name='sbw', bufs=2))

        nc.gpsimd.load_library(library_config.mlp)

        ident = sb.tile([128, 128], F32)
        make_identity(nc, ident[:])
        ident_bf = sb.tile([128, 128], BF16)
        nc.vector.tensor_copy(out=ident_bf[:], in_=ident[:])
        iota_r = sb.tile([128, 128], BF16)
        nc.sync.dma_start(out=iota_r[:], in_=inp['iota_row'][:, :])
        iota_c = sb.tile([128, 1], F32)
        nc.sync.dma_start(out=iota_c[:], in_=inp['iota_col'][:, :])
        ones_r = sb.tile([1, 128], F32)
        nc.sync.dma_start(out=ones_r[:], in_=inp['ones_row'][:, :])
        ones_bf = sb.tile([1, 128], BF16)
        nc.vector.memset(ones_bf[:], 1.0)

        W1e = [sb.tile([128, T1], BF16, tag=f'w1e{_i}', name=f'W1e{_i}') for _i in range(FH)]
        W2e = [sb.tile([128, T2], BF16, tag=f'w2e{_i}', name=f'W2e{_i}') for _i in range(FH)]
        b1b = sb.tile([128, D1], F32)
        b2b = sb.tile([128, CLS], F32)

        with tc.tile_pool(name='p0sb', bufs=1) as p0sb, \
             tc.tile_pool(name='p0ps', bufs=1, space='PSUM') as p0ps:
            for fh in range(FH):
                nc.vector.memset(W1e[fh][:], 0.0)
                nc.vector.memset(W2e[fh][:], 0.0)
            # ---- W1_ext = [W1cm | W1@A_src | W1@A_dst]
            a1 = p0sb.tile([H, 2 * C], F32)
            nc.sync.dma_start(out=a1[:, 0:C], in_=inp['a_src1'][:, :])
            nc.sync.dma_start(out=a1[:, C:2*C], in_=inp['a_dst1'][:, :])
            a1T_ps = p0ps.tile([128, 128], F32, space='PSUM', tag='t')
            nc.tensor.transpose(out=a1T_ps[0:2*C, 0:H], in_=a1[:], identity=ident[0:H, 0:H])
            a1T = p0sb.tile([2 * C, H], F32)
            nc.vector.tensor_copy(out=a1T[:], in_=a1T_ps[0:2*C, 0:H])
            A_bd = p0sb.tile([128, FH, 2 * H], F32)
            nc.vector.memset(A_bd[:], 0.0)
            for h in range(H):
                half, off = divmod(h * C, 128)
                nc.vector.tensor_copy(out=A_bd[off:off+C, half, h:h+1],
                                      in_=a1T[0:C, h:h+1])
                nc.vector.tensor_copy(out=A_bd[off:off+C, half, H+h:H+h+1],
                                      in_=a1T[C:2*C, h:h+1])
            w1t_sb = [p0sb.tile([128, F_IN], F32, tag=f'w1t{_i}', name=f'w1t{_i}') for _i in range(FH)]
            for cc in range(FH):
                nc.sync.dma_start(out=w1t_sb[cc][:],
                                  in_=inp['W1T'][cc*128:(cc+1)*128, :])
            w1a_ps = p0ps.tile([128, FH, 2 * H], F32, space='PSUM', tag='a')
            for fh in range(FH):
                for cc in range(FH):
                    nc.tensor.matmul(out=w1a_ps[:, fh, :],
                                     lhsT=w1t_sb[cc][:, fh*128:(fh+1)*128],
                                     rhs=A_bd[:, cc, :],
                                     start=(cc == 0), stop=(cc == FH - 1))
            for fh in range(FH):
                w1cm_t = p0sb.tile([128, D1], F32, tag='w1cm')
                nc.sync.dma_start(out=w1cm_t[:],
                                  in_=inp['W1cm'][fh*128:(fh+1)*128, :])
                nc.vector.tensor_copy(out=W1e[fh][:, 0:D1], in_=w1cm_t[:])
                nc.vector.tensor_copy(out=W1e[fh][:, D1:D1+2*H],
                                      in_=w1a_ps[:, fh, :])

            # ---- W2_ext = [W2p | W2p@a_src2^T | W2p@a_dst2^T]
            a2 = p0sb.tile([2, CLS], F32)
            nc.sync.dma_start(out=a2[0:1, :], in_=inp['a_src2'][:, :])
            nc.sync.dma_start(out=a2[1:2, :], in_=inp['a_dst2'][:, :])
            a2T_ps = p0ps.tile([128, 128], F32, space='PSUM', tag='t')
            nc.tensor.transpose(out=a2T_ps[0:CLS, 0:2], in_=a2[:], identity=ident[0:2, 0:2])
            a2T = p0sb.tile([CLS, 2], F32)
            nc.vector.tensor_copy(out=a2T[:], in_=a2T_ps[0:CLS, 0:2])
            w2t_sb = p0sb.tile([CLS, D1], F32)
            nc.sync.dma_start(out=w2t_sb[:], in_=inp['W2pT'][:, :])
            w2a_ps = p0ps.tile([128, FH, 2], F32, space='PSUM', tag='a')
            for fh in range(FH):
                nc.tensor.matmul(out=w2a_ps[:, fh, :],
                                 lhsT=w2t_sb[:, fh*128:(fh+1)*128],
                                 rhs=a2T[:], start=True, stop=True)
            for fh in range(FH):
                w2p_t = p0sb.tile([128, CLS], F32, tag='w2p')
                nc.sync.dma_start(out=w2p_t[:],
                                  in_=inp['W2p'][fh*128:(fh+1)*128, :])
                nc.vector.tensor_copy(out=W2e[fh][:, 0:CLS], in_=w2p_t[:])
                nc.vector.tensor_copy(out=W2e[fh][:, CLS:CLS+2],
                                      in_=w2a_ps[:, fh, :])

            # ---- bias broadcast tiles
            b1_sb = p0sb.tile([1, D1], F32)
            nc.sync.dma_start(out=b1_sb[:], in_=inp['b1cm'][:, :])
            b1b_ps = p0ps.tile([128, D1], F32, space='PSUM', tag='b')
            nc.tensor.matmul(out=b1b_ps[:], lhsT=ones_r[:], rhs=b1_sb[:],
                             start=True, stop=True)
            nc.vector.tensor_copy(out=b1b[:], in_=b1b_ps[:])
            b2_sb = p0sb.tile([1, CLS], F32)
            nc.sync.dma_start(out=b2_sb[:], in_=inp['b2'][:, :])
            b2b_ps = p0ps.tile([128, CLS], F32, space='PSUM', tag='b')
            nc.tensor.matmul(out=b2b_ps[:], lhsT=ones_r[:], rhs=b2_sb[:],
                             start=True, stop=True)
            nc.vector.tensor_copy(out=b2b[:], in_=b2b_ps[:])

        # ---- P1: z1 table build (rotated order)
        E1W = E1   # written cols (z | asrc | adst)
        T4 = 4      # node tiles per batched x load
        with tc.tile_pool(name='p1sb', bufs=3) as p1sb, \
             tc.tile_pool(name='p1ps', bufs=4, space='PSUM') as p1ps:
            for t4 in range(0, cfg.NP // 128, T4):
                xtb4 = []
                for fh in range(FH):
                    xt = p1sb.tile([128, T4 * 128], F32, tag=f'xt{fh}')
                    nc.sync.dma_start(
                        out=xt[:],
                        in_=inp['xT_rot'][fh*128:(fh+1)*128,
                                          t4*128:(t4+T4)*128])
                    xtb = p1sb.tile([128, T4 * 128], BF16, tag=f'xtb{fh}')
                    nc.vector.tensor_copy(out=xtb[:], in_=xt[:])
                    xtb4.append(xtb)
                for k in range(T4):
                    t = t4 + k
                    zps = p1ps.tile([128, E1W], F32, space='PSUM')
                    for fh in range(FH):
                        nc.tensor.matmul(out=zps[:],
                                         lhsT=xtb4[fh][:, k*128:(k+1)*128],
                                         rhs=W1e[fh][:, 0:E1W],
                                         start=(fh == 0), stop=(fh == FH - 1))
                    zsb = p1sb.tile([128, E1W], BF16, tag='zsb')
                    if t % 2 == 0:
                        nc.scalar.activation(zsb[:], zps[:],
                                             mybir.ActivationFunctionType.Copy)
                    else:
                        nc.vector.tensor_copy(out=zsb[:], in_=zps[:])
                    nc.scalar.dma_start(out=z1tab[t*128:(t+1)*128, 0:E1W],
                                        in_=zsb[:])

        # ---- edge phase (shared between layers)
        def edge_phase(layer, tab_lo, tab_hi, tabw, zcols, heads,
                       adst_ap, finalize):
            nmax = counts[layer]
            metarow_d = inp[f'metarow{layer}']
            tot = int(nmax.sum())

            with tc.tile_pool(name=f'ep{layer}', bufs=1) as ep, \
                 tc.tile_pool(name=f'em{layer}', bufs=4) as em, \
                 tc.tile_pool(name=f'eg{layer}', bufs=6) as eg, \
                 tc.tile_pool(name=f'ew{layer}', bufs=4) as ew, \
                 tc.tile_pool(name=f'el{layer}', bufs=2, space='PSUM') as el, \
                 tc.tile_pool(name=f'ea{layer}', bufs=2, space='PSUM') as ea:

                idx_lo = ep.tile([128, inp[f'idxlo{layer}'].shape[1]], I16)
                nc.sync.dma_start(out=idx_lo[:], in_=inp[f'idxlo{layer}'][:, :])
                idx_hi = ep.tile([128, inp[f'idxhi{layer}'].shape[1]], I16)
                nc.sync.dma_start(out=idx_hi[:], in_=inp[f'idxhi{layer}'][:, :])
                metacol = ep.tile([128, tot], BF16)
                nc.sync.dma_start(out=metacol[:], in_=inp[f'metacol{layer}'][:, :])

                gtiles = [{}, {}]
                stream_pos = [0, 0]
                mtiles = {}
                chunk_idx = 0
                for i in range(cfg.BPC):
                    nblk = int(nmax[i, 0] + nmax[i, 1])
                    if nblk == 0:
                        continue
                    adst_blk = ew.tile([128, heads], BF16, tag='adst')
                    nc.sync.dma_start(out=adst_blk[:], in_=adst_ap(i))
                    accps = ea.tile([128, zcols + heads], F32, space='PSUM',
                                    tag='acc')
                    done = 0
                    for g in range(2):
                        for _k in range(int(nmax[i, g])):
                            pos = stream_pos[g]
                            stream_pos[g] += 1
                            gi, j = divmod(pos, GG)
                            if gi not in gtiles[g]:
                                gt = eg.tile([128, GG, tabw], BF16, tag=f'g{g}')
                                nc.gpsimd.dma_gather(
                                    out_ap=gt[:],
                                    in_ap=tab_lo if g == 0 else tab_hi,
                                    idxs_ap=(idx_lo if g == 0 else idx_hi)[
                                        :, gi*64:(gi+1)*64],
                                    num_idxs=GG * BLK, num_idxs_reg=GG * BLK,
                                    elem_size=tabw)
                                gtiles[g][gi] = gt
                                for old in [k_ for k_ in gtiles[g]
                                            if k_ < gi - 2]:
                                    del gtiles[g][old]
                            gt = gtiles[g][gi]

                            mg, mj = divmod(chunk_idx, MG)
                            if mg not in mtiles:
                                lo = mg * MG * BLK
                                hi = min(tot * BLK, lo + MG * BLK)
                                mrow = em.tile([1, MG * BLK], BF16, tag='mrow')
                                nc.sync.dma_start(out=mrow[0:1, 0:hi-lo],
                                                  in_=metarow_d[0:1, lo:hi])
                                mb_ps = el.tile([128, MG * BLK], F32,
                                                space='PSUM', tag='mb')
                                nc.tensor.matmul(out=mb_ps[:, 0:hi-lo],
                                                 lhsT=ones_bf[:],
                                                 rhs=mrow[0:1, 0:hi-lo],
                                                 start=True, stop=True)
                                ohT_b = em.tile([128, MG * BLK], BF16,
                                                tag='ohtb')
                                nc.vector.tensor_scalar(
                                    out=ohT_b[:, 0:hi-lo],
                                    in0=mb_ps[:, 0:hi-lo],
                                    scalar1=iota_c[:, 0:1],
                                    scalar2=None, op0=mybir.AluOpType.is_equal)
                                nch = (hi - lo) // BLK
                                oh_b = em.tile([128, MG * BLK], BF16,
                                               tag='ohb')
                                nc.vector.tensor_tensor(
                                    out=oh_b[:, 0:hi-lo],
                                    in0=metacol[:, mg*MG:mg*MG+nch, None]
                                        .to_broadcast([128, nch, BLK]),
                                    in1=iota_r[:, None, :]
                                        .to_broadcast([128, nch, BLK]),
                                    op=mybir.AluOpType.is_equal)
                                mtiles[mg] = (ohT_b, oh_b)
                                for old in [k_ for k_ in mtiles
                                            if k_ < mg - 1]:
                                    del mtiles[old]
                            ohT_b, oh_b = mtiles[mg]
                            onehot = oh_b[:, mj*BLK:(mj+1)*BLK]

                            lg_ps = el.tile([128, heads], F32, space='PSUM',
                                            tag='lg')
                            nc.tensor.matmul(out=lg_ps[:],
                                             lhsT=ohT_b[:, mj*BLK:(mj+1)*BLK],
                                             rhs=adst_blk[:],
                                             start=True, stop=False)
                            nc.tensor.matmul(out=lg_ps[:], lhsT=ident_bf[:],
                                             rhs=gt[:, j, zcols:zcols+heads],
                                             start=False, stop=True)
                            # exp(lrelu(x)) == max(exp(x), exp(0.2x))
                            e1 = ew.tile([128, heads], F32, tag='e1')
                            nc.scalar.activation(e1[:], lg_ps[:],
                                                 mybir.ActivationFunctionType.Exp)
                            e2 = ew.tile([128, heads], F32, tag='e2')
                            nc.scalar.activation(e2[:], lg_ps[:],
                                                 mybir.ActivationFunctionType.Exp,
                                                 scale=NEG)
                            w_t = ew.tile([128, heads], BF16, tag='wt')
                            nc.vector.tensor_tensor(
                                out=w_t[:], in0=e1[:], in1=e2[:],
                                op=mybir.AluOpType.max)

                            smsg = ew.tile([128, zcols], BF16, tag='smsg')
                            nc.vector.tensor_tensor(
                                out=smsg[:], in0=gt[:, j, 0:zcols],
                                in1=w_t[:, None, :].to_broadcast(
                                    [128, zcols // heads, heads]),
                                op=mybir.AluOpType.mult)

                            nc.tensor.matmul(out=accps[:, 0:zcols],
                                             lhsT=onehot, rhs=smsg[:],
                                             start=(done == 0), stop=False)
                            nc.tensor.matmul(out=accps[:, zcols:zcols+heads],
                                             lhsT=onehot, rhs=w_t[:],
                                             start=False,
                                             stop=(done == nblk - 1))
                            done += 1
                            chunk_idx += 1
                    finalize(i, accps)

        # ---- L1 finalize: normalize + bias + elu -> h1loc
        def fin1(i, accps):
            den = sbw.tile([128, H], F32, tag='den')
            nc.vector.tensor_scalar(out=den[:], in0=accps[:, D1:D1+H],
                                    scalar1=1e-30, scalar2=None,
                                    op0=mybir.AluOpType.max)
            rec = sbw.tile([128, H], F32, tag='rec')
            nc.vector.reciprocal(out=rec[:], in_=den[:])
            h1t = sbw.tile([128, D1], F32, tag='h1t')
            nc.vector.tensor_tensor(
                out=h1t[:], in0=accps[:, 0:D1],
                in1=rec[:, None, :].to_broadcast([128, D1 // H, H]),
                op=mybir.AluOpType.mult)
            h1c = sbw.tile([128, D1], F32, tag='h1c')
            nc.vector.tensor_tensor(out=h1c[:], in0=h1t[:], in1=b1b[:],
                                    op=mybir.AluOpType.add)
            # elu(x) = exp(min(x,0)) - 1 + max(x,0)
            m0 = sbw.tile([128, D1], F32, tag='m0')
            nc.vector.tensor_scalar(out=m0[:], in0=h1c[:], scalar1=0.0,
                                    scalar2=None, op0=mybir.AluOpType.min)
            ex = sbw.tile([128, D1], F32, tag='ex')
            nc.scalar.activation(ex[:], m0[:], mybir.ActivationFunctionType.Exp)
            rl = sbw.tile([128, D1], F32, tag='rl')
            nc.vector.tensor_scalar(out=rl[:], in0=h1c[:], scalar1=0.0,
                                    scalar2=None, op0=mybir.AluOpType.max)
            h1f = sbw.tile([128, D1], F32, tag='h1f')
            nc.vector.tensor_tensor(out=h1f[:], in0=ex[:], in1=rl[:],
                                    op=mybir.AluOpType.add)
            h1o = sbw.tile([128, D1], F32, tag='h1o')
            nc.vector.tensor_scalar(out=h1o[:], in0=h1f[:], scalar1=-1.0,
                                    scalar2=None, op0=mybir.AluOpType.add)
            nc.sync.dma_start(out=h1loc[i*128:(i+1)*128, :], in_=h1o[:])

        edge_phase(1, z1tab[0:cfg.HALF, :], z1tab[cfg.HALF:, :], T1, D1, H,
                   lambda i: z1tab[i*128:(i+1)*128, D1+H:D1+2*H], fin1)

        # ---- P3: z2slice = [h1 @ W2p | asrc2 | adst2]
        E2 = CLS + 2
        with tc.tile_pool(name='p3sb', bufs=3) as p3sb, \
             tc.tile_pool(name='p3ps', bufs=2, space='PSUM') as p3ps:
            for t in range(cfg.BPC):
                h1tile = p3sb.tile([128, D1], F32, tag='h1')
                nc.sync.dma_start(out=h1tile[:], in_=h1loc[t*128:(t+1)*128, :])
                zps = p3ps.tile([128, E2], F32, space='PSUM', tag='z')
                for fh in range(FH):
                    tp = p3ps.tile([128, 128], F32, space='PSUM', tag='tp')
                    nc.tensor.transpose(out=tp[:],
                                        in_=h1tile[:, fh*128:(fh+1)*128],
                                        identity=ident[:])
                    h1T = p3sb.tile([128, 128], BF16, tag='h1T')
                    if fh % 2 == 0:
                        nc.vector.tensor_copy(out=h1T[:], in_=tp[:])
                    else:
                        nc.scalar.activation(h1T[:], tp[:],
                                             mybir.ActivationFunctionType.Copy)
                    nc.tensor.matmul(out=zps[:], lhsT=h1T[:],
                                     rhs=W2e[fh][:, 0:E2],
                                     start=(fh == 0), stop=(fh == FH - 1))
                zsb = p3sb.tile([128, E2], BF16, tag='zsb')
                nc.vector.tensor_copy(out=zsb[:], in_=zps[:])
                nc.sync.dma_start(out=z2slice[t*128:(t+1)*128, 0:E2], in_=zsb[:])

        # ---- P4: AllGather z2slice -> z2cat
        nc.gpsimd.collective_compute(
            'AllGather', mybir.AluOpType.bypass,
            replica_groups=[list(range(cfg.CORES))],
            ins=[z2slice.ap().opt()],
            outs=[z2cat.ap().opt()])

        # ---- L2 finalize: normalize + bias -> out
        def fin2(i, accps):
            den = sbw.tile([128, 1], F32, tag='den2')
            nc.vector.tensor_scalar(out=den[:], in0=accps[:, CLS:CLS+1],
                                    scalar1=1e-30, scalar2=None,
                                    op0=mybir.AluOpType.max)
            rec2 = sbw.tile([128, 1], F32, tag='rec2')
            nc.vector.reciprocal(out=rec2[:], in_=den[:])
            o1 = sbw.tile([128, CLS], F32, tag='o1')
            nc.vector.tensor_scalar(out=o1[:], in0=accps[:, 0:CLS],
                                    scalar1=rec2[:, 0:1], scalar2=None,
                                    op0=mybir.AluOpType.mult)
            o2 = sbw.tile([128, CLS], F32, tag='o2')
            nc.vector.tensor_tensor(out=o2[:], in0=o1[:], in1=b2b[:],
                                    op=mybir.AluOpType.add)
            nc.sync.dma_start(out=out_d[i*128:(i+1)*128, :], in_=o2[:])

        edge_phase(2, z2cat[0:cfg.HALF, :], z2cat[cfg.HALF:, :], T2, CLS, 1,
                   lambda i: z2slice[i*128:(i+1)*128, CLS+1:CLS+2], fin2)

    return nc


# ---------------------------------------------------------------- entry point

def run(inputs, cfg, sim=False):
    counts, per_core = host_prep(inputs['edge_index'], cfg)
    consts, xT_rots = host_weights(inputs, cfg)
    nc = build_gat(counts, cfg)
    in_maps = []
    for c in range(cfg.CORES):
        m = dict(consts)
        m['xT_rot'] = xT_rots[c]
        m.update(per_core[c])
        in_maps.append(m)
    if not sim:
        nc.compile()
    if sim:
        from concourse import bass_interp
        ms = bass_interp.MultiCoreSim(nc, cfg.CORES,
                                      num_workers=min(8, cfg.CORES))
        for c in range(cfg.CORES):
            for k, v in in_maps[c].items():
                ms.cores[c].tensor(k)[:] = v
        ms.simulate()
        outs = [np.array(ms.cores[c].mem_tensor('out')).reshape(cfg.SLICE, cfg.CLS)
                for c in range(cfg.CORES)]
    else:
        res = run_bass_kernel_spmd(nc, in_maps, core_ids=list(range(cfg.CORES)))
        outs = [np.asarray(res.results[c]['out']).reshape(cfg.SLICE, cfg.CLS)
                for c in range(cfg.CORES)]
    full = np.concatenate(outs, 0)
    return full[:cfg.N].astype(np.float32)


def kernel(**inputs):
    return run(inputs, FULL, sim=False)

